# revision 34
# baseline (speedup 1.0000x reference)
"""Trainium2 Bass kernel for nn_Attention (T=2048, D=2048, H=16, Dh=128).

Strategy: tensor-parallel over heads, 2 heads per core on 8 cores.

v2: fp8 (e4m3) DoubleRow matmuls with full error compensation.
  - host folds w_ln and the per-token RMSNorm scale s into the RoPE tables
    and the v scale (the same class of input prep as the existing w_ln /
    sign folding); device does all matmul + attention compute
  - projections: x and w split hi/lo fp8 (x = x_hi + x_lo/16, w = w_hi +
    w_lo/16); three DoubleRow chains (hi*hi at x32, hi*lo + lo*hi at x512)
    accumulated in two PSUM groups, combined by one scalar_tensor_tensor;
    the 1/512 descale is folded into the host-side RoPE tables / v scale.
    Net: ~bf16 accuracy at 75% of the bf16 matmul cost on PE.
  - attention: transposed-score layout (S^T[tk,tq]); causal masking via a
    -1e9 bias add on the score PSUM before exp (Pool engine); strip j=0
    (tq<512) keeps bf16 probs; strips 1-3 exp to fp8 with a -4.5 offset
    (cancelled by the deferred softmax normalization) and run PV + rowsum
    as fp8 DoubleRow over tk-tile pairs with bf16-accuracy margins
  - wo: bf16 matmuls, heads accumulated in PSUM; output staged bf16
    (partials summed on host in f32)
"""

import math
import os
import sys
import time

for _p in ("/opt/trn_rl_repo", "/root/.axon_site/_ro/trn_rl_repo"):
    if os.path.isdir(_p) and _p not in sys.path:
        sys.path.insert(0, _p)

import numpy as np
import ml_dtypes

import concourse.bass as bass
import concourse.tile as tile
from concourse import bacc, mybir
from concourse.bass_utils import run_bass_kernel_spmd

BF16 = mybir.dt.bfloat16
F8 = mybir.dt.float8e4
F32R = mybir.dt.float32r
F32 = mybir.dt.float32
AF = mybir.ActivationFunctionType
ALU = mybir.AluOpType
DR = mybir.MatmulPerfMode.DoubleRow

T = 2048
D = 2048
N_H = 16
D_H = 128
N_CORES = 8
H_LOC = N_H // N_CORES          # heads per core = 2
NL = H_LOC * D_H                # local head width = 256
KD = D // 128                   # contraction tiles = 16
KP = KD // 2                    # DoubleRow contraction pairs = 8
TT = T // 128                   # t tiles = 16
NS = T // 512                   # 512-wide strips = 4
EPS = 1e-5
INV_SQRT_DH = 1.0 / math.sqrt(D_H)
EOFF = 4.5                      # fp8 exp offset (cancels in normalization)
WSC = 32.0                      # fp8 weight scale (hi and lo residual alike)
OSC = 512.0                     # output scale (16 from v, 32 from wo weights)

_CACHED = {}
_MARKS = []          # (label, next-instruction-name) pairs when _MARK_ON
_MARK_ON = [False]   # analysis hook; stays off in normal runs


def _build_program(repeats=1):
    if repeats in _CACHED:
        return _CACHED[repeats]

    nc = bacc.Bacc("TRN2", target_bir_lowering=False, debug=False, num_devices=N_CORES)

    # all bulk inputs host-prepacked to partition-major [128, ...] so every
    # DMA is 128 long contiguous runs
    xhi_ds = [nc.dram_tensor(f"xhi{j}", [128, KD * 512], F8, kind="ExternalInput")
              for j in range(NS)]
    xlo_ds = [nc.dram_tensor(f"xlo{j}", [128, KD * 512], F8, kind="ExternalInput")
              for j in range(NS)]
    wq_hi_d = nc.dram_tensor("wqhi", [128, KD * NL], F8, kind="ExternalInput")
    wq_lo_d = nc.dram_tensor("wqlo", [128, KD * NL], F8, kind="ExternalInput")
    wk_hi_d = nc.dram_tensor("wkhi", [128, KD * NL], F8, kind="ExternalInput")
    wk_lo_d = nc.dram_tensor("wklo", [128, KD * NL], F8, kind="ExternalInput")
    wv_hi_d = nc.dram_tensor("wvhi", [128, KD * NL], F8, kind="ExternalInput")
    wv_lo_d = nc.dram_tensor("wvlo", [128, KD * NL], F8, kind="ExternalInput")
    wo_hi_d = nc.dram_tensor("wohi", [128, H_LOC * T], F8, kind="ExternalInput")
    wo_lo_d = nc.dram_tensor("wolo", [128, H_LOC * T], F8, kind="ExternalInput")
    cos_d = nc.dram_tensor("cosT", [D_H, T], BF16, kind="ExternalInput")
    sin_d = nc.dram_tensor("sinT", [D_H, T], BF16, kind="ExternalInput")
    skt_d = nc.dram_tensor("skt", [128, TT], F32, kind="ExternalInput")
    mskb_d = nc.dram_tensor("maskb", [128, 128], BF16, kind="ExternalInput")
    onc_d = nc.dram_tensor("ones_col", [1, 128], F32R, kind="ExternalInput")
    on128_d = nc.dram_tensor("ones128", [128, 1], BF16, kind="ExternalInput")
    on2_d = nc.dram_tensor("ones2f8", [128, 128], F8, kind="ExternalInput")
    out_d = nc.dram_tensor("out", [T, D], BF16, kind="ExternalOutput")

    ap = lambda h: h.ap()
    out_ap = ap(out_d)

    from contextlib import ExitStack

    with tile.TileContext(nc) as tc, ExitStack() as ctx:
        P = ctx.enter_context  # noqa

        singles = P(tc.tile_pool(name="singles", bufs=1))
        rope = P(tc.tile_pool(name="rope", bufs=4))        # [128,512] bf16
        epool = P(tc.tile_pool(name="epool", bufs=10))     # [128,2,512] f8 pairs
        e0pool = P(tc.tile_pool(name="e0pool", bufs=5))    # [128,512] bf16 strip0
        rbsp = P(tc.tile_pool(name="rbsp", bufs=2))        # [128,512] bf16 pv evac
        small = P(tc.tile_pool(name="small", bufs=2))      # [1,512] f32
        stage = P(tc.tile_pool(name="stage", bufs=4))      # [128,T] bf16 out staging
        pA = P(tc.tile_pool(name="pA", bufs=3, space="PSUM"))   # proj + wo
        psc = P(tc.tile_pool(name="psc", bufs=3, space="PSUM")) # scores + tail wo
        ppv = P(tc.tile_pool(name="ppv", bufs=1, space="PSUM")) # po + rb
        psu = P(tc.tile_pool(name="psu", bufs=1, space="PSUM")) # su

        def mark(label):
            if _MARK_ON[0]:
                _MARKS.append((label, nc.get_next_instruction_name()))

        def emit_body():
            mark("loads")
            # ---------------- phase 0: loads in consumption order ------------------
            # PE warmup: ~5us of tiny matmuls during the initial DMA wait so the
            # p-state ramp completes before real work arrives
            warm = singles.tile([128, 256], BF16, tag="warm")
            nc.gpsimd.memset(warm, 1.0)
            eoffb = singles.tile([128, 1], F32, tag="eoffb")
            nc.vector.memset(eoffb, -EOFF)
            for _w in range(16):
                pw = psc.tile([128, 256], F32, tag="sc", name=f"warm{_w}")
                nc.tensor.matmul(pw, lhsT=warm[:, 0:128], rhs=warm,
                                 start=True, stop=True)

            xhi_t = [
                singles.tile([128, KD, 512], F8, tag=f"xhi{j}", name=f"xhi{j}")
                for j in range(NS)
            ]
            xlo_t = [
                singles.tile([128, KD, 512], F8, tag=f"xlo{j}", name=f"xlo{j}")
                for j in range(NS)
            ]

            def load_x_strip(j, hi, half=None):
                t_, d_ = (xhi_t[j], xhi_ds[j]) if hi else (xlo_t[j], xlo_ds[j])
                dv = ap(d_).rearrange("p (a m) -> p a m", a=KD)
                if half is None:
                    nc.sync.dma_start(out=t_, in_=dv)
                elif half == 0:
                    nc.sync.dma_start(out=t_[:, 0:8, :], in_=dv[:, 0:8, :])
                else:
                    nc.sync.dma_start(out=t_[:, 8:16, :], in_=dv[:, 8:16, :])

            def load_w(dram, tag, split=False):
                t_ = singles.tile([128, KD, NL], F8, tag=tag)
                dv = ap(dram).rearrange("p (a m) -> p a m", a=KD)
                if split:
                    nc.sync.dma_start(out=t_[:, 0:8, :], in_=dv[:, 0:8, :])
                    nc.sync.dma_start(out=t_[:, 8:16, :], in_=dv[:, 8:16, :])
                else:
                    nc.sync.dma_start(out=t_, in_=dv)
                return t_

            # interleave x0/w halves so the first chains start ~3us in
            wkh = singles.tile([128, KD, NL], F8, tag="wkh")
            wkhv = ap(wk_hi_d).rearrange("p (a m) -> p a m", a=KD)
            nc.sync.dma_start(out=wkh[:, 0:8, :], in_=wkhv[:, 0:8, :])
            load_x_strip(0, True, 0)
            nc.sync.dma_start(out=wkh[:, 8:16, :], in_=wkhv[:, 8:16, :])
            load_x_strip(0, True, 1)
            wkl = load_w(wk_lo_d, "wkl", split=True)
            load_x_strip(0, False, 0)
            load_x_strip(0, False, 1)
            wqh = load_w(wq_hi_d, "wqh", split=True)
            wql = load_w(wq_lo_d, "wql", split=True)
            wvh = load_w(wv_hi_d, "wvh")
            cos_s = singles.tile([128, T], BF16, tag="cos_s")
            nc.sync.dma_start(out=cos_s, in_=ap(cos_d))
            sin_s = singles.tile([128, T], BF16, tag="sin_s")
            nc.sync.dma_start(out=sin_s, in_=ap(sin_d))
            wvl = load_w(wv_lo_d, "wvl")
            sk_t = singles.tile([128, TT], F32, tag="sk")
            nc.sync.dma_start(out=sk_t, in_=ap(skt_d))
            for j in range(1, NS):
                load_x_strip(j, True)
                load_x_strip(j, False)
            del wkhv
            mskb = singles.tile([128, 128], BF16, tag="mskb")
            nc.sync.dma_start(out=mskb, in_=ap(mskb_d))
            onc = singles.tile([1, 128], F32R, tag="onc")
            nc.sync.dma_start(out=onc, in_=ap(onc_d))
            on128 = singles.tile([128, 1], BF16, tag="on128")
            nc.sync.dma_start(out=on128, in_=ap(on128_d))
            on2 = singles.tile([128, 2, 64], F8, tag="on2")
            nc.sync.dma_start(out=on2, in_=ap(on2_d).rearrange("p (a b) -> p a b", a=2))
            woh = singles.tile([128, H_LOC, T], F8, tag="woh")
            nc.sync.dma_start(out=woh, in_=ap(wo_hi_d).rearrange("p (h t) -> p h t", h=H_LOC))
            wol = singles.tile([128, H_LOC, T], F8, tag="wol")
            nc.sync.dma_start(out=wol, in_=ap(wo_lo_d).rearrange("p (h t) -> p h t", h=H_LOC))

            # ---------------- projections ------------------------------------------
            q_sb = singles.tile([128, H_LOC, T], BF16, tag="q_sb")
            k_sb = singles.tile([128, H_LOC, T], BF16, tag="k_sb")
            v8 = singles.tile([128, TT, NL], F8, tag="v8")
            v0 = singles.tile([128, 4, NL], BF16, tag="v0")
            outT = singles.tile([128, H_LOC, T], BF16, tag="outT")
            ohi = singles.tile([128, H_LOC, T], F8, tag="ohi")
            olo = singles.tile([128, H_LOC, T], F8, tag="olo")

            def emit_proj_psum(whi, wlo, h, j):
                """Single-group DoubleRow projection at x32: hi*hi + hi*lo +
                lo*hi (residuals stored at the same scale as hi)."""
                hs = slice(h * 128, (h + 1) * 128)
                xh, xl = xhi_t[j], xlo_t[j]
                ps = pA.tile([128, 512], F32, tag="pa")
                for kp in range(KP):
                    ks = slice(2 * kp, 2 * kp + 2)
                    nc.tensor.matmul(
                        ps, lhsT=whi[:, ks, hs], rhs=xh[:, ks, :],
                        start=(kp == 0), stop=False, perf_mode=DR,
                    )
                for kp in range(KP):
                    ks = slice(2 * kp, 2 * kp + 2)
                    nc.tensor.matmul(
                        ps, lhsT=whi[:, ks, hs], rhs=xl[:, ks, :],
                        start=False, stop=False, perf_mode=DR,
                    )
                for kp in range(KP):
                    ks = slice(2 * kp, 2 * kp + 2)
                    nc.tensor.matmul(
                        ps, lhsT=wlo[:, ks, hs], rhs=xh[:, ks, :],
                        start=False, stop=(kp == KP - 1), perf_mode=DR,
                    )
                return ps

            def emit_qk_strip(h, j):
                mark(f"qk h{h} j{j}")
                js = slice(j * 512, (j + 1) * 512)
                for dst, whi, wlo in ((k_sb, wkh, wkl), (q_sb, wqh, wql)):
                    ps = emit_proj_psum(whi, wlo, h, j)
                    # RoPE: rotate-half via a partition-swapping DMA (engines
                    # cannot cross partitions); tables carry s/32 and the sign
                    m1 = rope.tile([128, 512], BF16, tag="m1")
                    nc.vector.tensor_mul(m1, ps, cos_s[:, js])
                    m2 = rope.tile([128, 512], BF16, tag="m2")
                    nc.vector.tensor_mul(m2[0:64, :], ps[64:128, :], sin_s[0:64, js])
                    nc.vector.tensor_mul(m2[64:128, :], ps[0:64, :], sin_s[64:128, js])
                    nc.vector.tensor_add(dst[:, h, js], m1, m2)

            def emit_v_tile(tt):
                # v in [tk, dh] layout directly: lhsT = x (tk columns as the
                # stationary free dim), rhs = wv; no transpose needed.
                mark(f"v tt{tt}")
                j, lt = tt // 4, tt % 4
                ls = slice(lt * 128, (lt + 1) * 128)
                xh, xl = xhi_t[j], xlo_t[j]
                ps = pA.tile([128, NL], F32, tag="pa", name="vps")
                for kp in range(KP):
                    ks = slice(2 * kp, 2 * kp + 2)
                    nc.tensor.matmul(
                        ps, lhsT=xh[:, ks, ls], rhs=wvh[:, ks, :],
                        start=(kp == 0), stop=False, perf_mode=DR,
                    )
                for kp in range(KP):
                    ks = slice(2 * kp, 2 * kp + 2)
                    nc.tensor.matmul(
                        ps, lhsT=xl[:, ks, ls], rhs=wvh[:, ks, :],
                        start=False, stop=False, perf_mode=DR,
                    )
                for kp in range(KP):
                    ks = slice(2 * kp, 2 * kp + 2)
                    nc.tensor.matmul(
                        ps, lhsT=xh[:, ks, ls], rhs=wvl[:, ks, :],
                        start=False, stop=(kp == KP - 1), perf_mode=DR,
                    )
                nc.scalar.activation(v8[:, tt, :], ps, AF.Copy,
                                     scale=sk_t[:, tt : tt + 1])
                if tt < 4:
                    nc.vector.tensor_scalar_mul(v0[:, tt, :], ps, sk_t[:, tt : tt + 1])

            # ---------------- attention --------------------------------------------
            # A phase: all scores + exps of a strip (PE streams scores while
            # Pool/ACT chase with mask + exp). B phase: PV + rowsum + epilogue
            # (runs with a full strip of slack behind the exps).
            e_units = {}
            fin = {}

            def emit_att_A(h, j, fillers=()):
                mark(f"attA h{h} j{j}")
                fillers = list(fillers)
                nunit = 0

                def tick():
                    nonlocal nunit
                    nunit += 1
                    if nunit % 2 == 0 and fillers:
                        fillers.pop(0)()

                js = slice(j * 512, (j + 1) * 512)
                units = []
                if j == 0:
                    for i in range(4):
                        c0 = 128 * i
                        cs = slice(c0, 512)
                        st = psc.tile([128, 512], F32, tag="sc")
                        nc.tensor.matmul(
                            st[:, cs], lhsT=k_sb[:, h, i * 128 : (i + 1) * 128],
                            rhs=q_sb[:, h, cs], start=True, stop=True,
                        )
                        e0 = e0pool.tile([128, 512], BF16, tag="e0")
                        nc.scalar.activation(e0[:, cs], st[:, cs], AF.Exp,
                                             scale=INV_SQRT_DH)
                        nc.gpsimd.tensor_mul(
                            e0[:, c0 : c0 + 128], e0[:, c0 : c0 + 128], mskb
                        )
                        units.append((e0, c0))
                        tick()
                else:
                    npair = 2 * (j + 1)
                    for p_ in range(npair):
                        i0 = 2 * p_
                        r0 = i0 - 4 * j
                        c0p = 128 * r0 if r0 >= 0 else 0
                        e8 = epool.tile([128, 2, 512], F8, tag="e8")
                        for slot in range(2):
                            i = i0 + slot
                            r = i - 4 * j
                            c0 = 128 * r if r > 0 else 0
                            cs = slice(c0, 512)
                            qs = slice(j * 512 + c0, (j + 1) * 512)
                            st = psc.tile([128, 512], F32, tag="sc")
                            nc.tensor.matmul(
                                st[:, cs], lhsT=k_sb[:, h, i * 128 : (i + 1) * 128],
                                rhs=q_sb[:, h, qs], start=True, stop=True,
                            )
                            nc.scalar.activation(e8[:, slot, cs], st[:, cs], AF.Exp,
                                                 scale=INV_SQRT_DH, bias=eoffb)
                            if r >= 0:
                                nc.gpsimd.tensor_mul(
                                    e8[:, slot, c0 : c0 + 128],
                                    e8[:, slot, c0 : c0 + 128], mskb
                                )
                            if slot == 1 and c0 > c0p:
                                nc.gpsimd.memset(e8[:, 1, c0p:c0], 0.0)
                        units.append((e8, c0p))
                        tick()
                e_units[(h, j)] = units
                for f in fillers:
                    f()

            def emit_att_B(h, j):
                mark(f"attB h{h} j{j}")
                hs = slice(h * 128, (h + 1) * 128)
                js = slice(j * 512, (j + 1) * 512)
                units = e_units.pop((h, j))
                po = ppv.tile([128, 512], F32, tag="pv")
                su = psu.tile([64, 512], F32, tag="su")
                n = len(units)
                if j == 0:
                    for i, (e0, c0) in enumerate(units):
                        cs = slice(c0, 512)
                        nc.tensor.matmul(
                            po[:, cs], lhsT=v0[:, i, hs], rhs=e0[:, cs],
                            start=(i == 0), stop=(i == n - 1),
                        )
                        nc.tensor.matmul(
                            su[0:1, cs], lhsT=on128, rhs=e0[:, cs],
                            start=(i == 0), stop=(i == n - 1),
                        )
                else:
                    for p_, (e8, c0p) in enumerate(units):
                        csp = slice(c0p, 512)
                        i0 = 2 * p_
                        nc.tensor.matmul(
                            po[:, csp], lhsT=v8[:, i0 : i0 + 2, hs],
                            rhs=e8[:, :, csp],
                            start=(p_ == 0), stop=(p_ == n - 1), perf_mode=DR,
                        )
                        nc.tensor.matmul(
                            su[:, csp], lhsT=on2, rhs=e8[:, :, csp],
                            start=(p_ == 0), stop=(p_ == n - 1), perf_mode=DR,
                        )  # 64 identical rowsum rows; row 0 is used
                pos = rbsp.tile([128, 512], BF16, tag="pos")
                nc.vector.tensor_copy(pos, po)
                rec = small.tile([1, 512], F32R, tag="rec")
                with nc.allow_low_precision(reason="1/su feeds an f32r bcast"):
                    nc.vector.reciprocal(rec, su[0:1, :])
                fin[(h, j)] = (pos, rec)

            def emit_att_B2(h, j):
                mark(f"attB2 h{h} j{j}")
                js = slice(j * 512, (j + 1) * 512)
                pos, rec = fin.pop((h, j))
                rb = ppv.tile([128, 512], F32, tag="pv", name="rb")
                nc.tensor.matmul(rb, lhsT=onc, rhs=rec, start=True, stop=True)
                nc.vector.tensor_mul(outT[:, h, js], rb, pos)
                nc.gpsimd.tensor_copy(ohi[:, h, js], outT[:, h, js])
                nc.gpsimd.tensor_sub(olo[:, h, js], outT[:, h, js], ohi[:, h, js])

            def emit_wo_tile(tt, tail=False):
                # fp8 3-chain in one x512 PSUM group: (ohi+olo)*wohi +
                # (outT/16 as fp8)*wolo; host divides by 512
                mark(f"wo tt{tt}")
                ts = slice(tt * 128, (tt + 1) * 128)
                stg = stage.tile([128, T], BF16, tag="stg")
                for n in range(NS):
                    ns = slice(n * 512, (n + 1) * 512)
                    if tail and n % 2 == 1:
                        pa = psc.tile([128, 512], F32, tag="sc", name="wot")
                    else:
                        pa = pA.tile([128, 512], F32, tag="pa")
                    nc.tensor.matmul(
                        pa, lhsT=ohi[:, :, ts], rhs=woh[:, :, ns],
                        start=True, stop=False, perf_mode=DR,
                    )
                    nc.tensor.matmul(
                        pa, lhsT=olo[:, :, ts], rhs=woh[:, :, ns],
                        start=False, stop=False, perf_mode=DR,
                    )
                    nc.tensor.matmul(
                        pa, lhsT=ohi[:, :, ts], rhs=wol[:, :, ns],
                        start=False, stop=True, perf_mode=DR,
                    )
                    if n % 2 == 0:
                        nc.vector.tensor_copy(stg[:, ns], pa)
                    else:
                        nc.scalar.copy(stg[:, ns], pa)
                    if n == 1:
                        nc.sync.dma_start(out=out_ap[ts, 0:1024], in_=stg[:, 0:1024])
                nc.sync.dma_start(out=out_ap[ts, 1024:2048], in_=stg[:, 1024:2048])

            # emission order mirrors data readiness: attention strip j only
            # needs x strips <= j, so each strip's qk/v/attention pipeline
            # rides directly behind its x DMA; wo tiles and deferred
            # epilogues (B2) fill the ACT-paced score streams.
            wo_f = lambda tt: (lambda: emit_wo_tile(tt))
            b2_f = lambda h, j: (lambda: emit_att_B2(h, j))

            emit_qk_strip(0, 0)
            emit_qk_strip(1, 0)
            for tt in range(0, 4):
                emit_v_tile(tt)
            emit_att_A(0, 0)
            emit_att_B(0, 0)
            emit_att_A(1, 0, [b2_f(0, 0)])
            emit_att_B(1, 0)
            emit_qk_strip(0, 1)
            emit_qk_strip(1, 1)
            for tt in range(4, 8):
                emit_v_tile(tt)
            emit_att_A(0, 1, [b2_f(1, 0)])
            emit_att_B(0, 1)
            emit_att_A(1, 1, [b2_f(0, 1)])
            emit_att_B(1, 1)
            emit_qk_strip(0, 2)
            emit_qk_strip(1, 2)
            for tt in range(8, 12):
                emit_v_tile(tt)
            emit_att_A(0, 2, [b2_f(1, 1), wo_f(0), wo_f(1)])
            emit_att_B(0, 2)
            emit_att_A(1, 2, [b2_f(0, 2), wo_f(2), wo_f(3)])
            emit_att_B(1, 2)
            emit_qk_strip(0, 3)
            emit_qk_strip(1, 3)
            for tt in range(12, 16):
                emit_v_tile(tt)
            emit_att_A(0, 3, [b2_f(1, 2), wo_f(4), wo_f(5), wo_f(6)])
            emit_att_B(0, 3)
            emit_att_A(1, 3, [b2_f(0, 3), wo_f(7), wo_f(8), wo_f(9), wo_f(10),
                              wo_f(11)])
            emit_att_B(1, 3)
            emit_att_B2(1, 3)
            for tt in (12, 13, 14, 15):
                emit_wo_tile(tt, tail=True)

        for _rep in range(repeats):
            emit_body()

    # Force Exp and Ln onto the single combined table set (one
    # ACT_TABLE_LOAD for the whole kernel).
    from concourse.hw_specs import get_activation_tables
    tabs = get_activation_tables(nc.m.arch)
    for nm_, fs_ in tabs.items():
        if nm_ != "natural_log_exp_and_others":
            fs_.discard(AF.Exp)
            fs_.discard(AF.Ln)
    nc.compile()
    _CACHED[repeats] = nc
    return nc


def _host_prep(x, w_ln, wq, wk, wv, wo, cos, sin):
    bf = ml_dtypes.bfloat16
    f8 = ml_dtypes.float8_e4m3
    x = np.asarray(x, np.float32)
    w_ln = np.asarray(w_ln, np.float32)

    # per-token RMSNorm scale, folded into the RoPE tables and v scale
    s = 1.0 / np.sqrt((x * x).mean(axis=1) + EPS)          # [T] f32

    cosT = np.ascontiguousarray(np.asarray(cos, np.float32).T)   # [D_H, T]
    sinT = np.ascontiguousarray(np.asarray(sin, np.float32).T)
    sinT[0:64] *= -1.0          # rotate_half sign folded into the table
    cos_s = (cosT * (s / WSC)[None, :]).astype(bf)
    sin_s = (sinT * (s / WSC)[None, :]).astype(bf)
    # v8 carries x16 (for the fp8 wo split); proj psum carries x32
    skt = np.ascontiguousarray((s * 16.0 / WSC).reshape(TT, 128).T).astype(np.float32)

    xT = np.ascontiguousarray(x.T).astype(bf).astype(np.float32)
    x_hi = xT.astype(f8)
    x_lo = (xT - x_hi.astype(np.float32)).astype(f8)     # same scale as hi

    def pack_kd(a, ncols):
        # [D, M] -> [128, KD*M]: tile[p, kd*M + m] = a[kd*128 + p, m]
        return np.ascontiguousarray(
            a.reshape(KD, 128, ncols).transpose(1, 0, 2).reshape(128, KD * ncols))

    xhi_p = {f"xhi{j}": pack_kd(x_hi[:, j * 512:(j + 1) * 512], 512) for j in range(NS)}
    xlo_p = {f"xlo{j}": pack_kd(x_lo[:, j * 512:(j + 1) * 512], 512) for j in range(NS)}

    # causal boundary: 1 where tq >= tk within the tile, else 0
    f = np.arange(128)[None, :]
    p = np.arange(128)[:, None]
    maskb = (f >= p).astype(bf)

    ones_col = np.ones((1, 128), np.float32)
    ones128 = np.ones((128, 1), bf)
    ones2f8 = np.ones((128, 128), f8)

    wq_s = np.asarray(wq, np.float32) * w_ln[None, :]
    wk_s = np.asarray(wk, np.float32) * w_ln[None, :]
    wv_s = np.asarray(wv, np.float32) * w_ln[None, :]
    wo32 = np.asarray(wo, np.float32)

    def wsplit(w_sl):
        # [D, NL] slice, bf16-rounded like the reference weights path;
        # hi and the residual lo are stored at the same x32 scale
        wT = np.ascontiguousarray(w_sl.T).astype(bf).astype(np.float32) * WSC
        hi = wT.astype(f8)
        lo = (wT - hi.astype(np.float32)).astype(f8)
        return pack_kd(hi, NL), pack_kd(lo, NL)

    in_maps = []
    for c in range(N_CORES):
        sl = slice(c * NL, (c + 1) * NL)
        wqh, wql = wsplit(wq_s[sl])
        wkh, wkl = wsplit(wk_s[sl])
        wvh, wvl = wsplit(wv_s[sl])
        woT = np.ascontiguousarray(wo32[:, sl].T).astype(bf).astype(np.float32) * WSC
        woT_hi = woT.astype(f8)
        woT_lo = (woT - woT_hi.astype(np.float32)).astype(f8)

        def pack_h(a):
            return np.ascontiguousarray(
                a.reshape(H_LOC, 128, T).transpose(1, 0, 2).reshape(128, H_LOC * T))

        in_maps.append({
            **xhi_p, **xlo_p,
            "wqhi": wqh, "wqlo": wql,
            "wkhi": wkh, "wklo": wkl,
            "wvhi": wvh, "wvlo": wvl,
            "wohi": pack_h(woT_hi), "wolo": pack_h(woT_lo),
            "cosT": cos_s,
            "sinT": sin_s,
            "skt": skt,
            "maskb": maskb,
            "ones_col": ones_col,
            "ones128": ones128,
            "ones2f8": ones2f8,
        })
    return in_maps


def kernel(x, w_ln, wq, wk, wv, wo, cos, sin):
    nc = _build_program()
    in_maps = _host_prep(x, w_ln, wq, wk, wv, wo, cos, sin)
    t0 = time.time()
    res = run_bass_kernel_spmd(nc, in_maps, core_ids=list(range(N_CORES)))
    t1 = time.time()
    print(f"run_bass_kernel_spmd wall: {(t1 - t0) * 1e3:.1f} ms", file=sys.stderr)
    acc = np.zeros((T, D), np.float32)
    for r in res.results:
        acc += np.asarray(r["out"], np.float32)
    return np.asarray(x, np.float32) + acc * (1.0 / OSC)


# revision 51
# speedup vs baseline: 757.0467x; 757.0467x over previous
"""Trainium2 Bass kernel for nn_Attention (T=2048, D=2048, H=16, Dh=128).

Strategy: tensor-parallel over heads, 2 heads per core on 8 cores.
fp8 (e4m3) DoubleRow matmuls everywhere precision allows, with same-scale
residual compensation so the result keeps bf16-level accuracy:

  - host prep: w_ln and the per-token RMSNorm scale s fold into the RoPE
    tables / v scale; every operand is split into fp8 hi + residual lo at
    the SAME scale (w at x32, x at x1) and prepacked partition-major so
    all DMAs are 128 long contiguous runs
  - projections q/k/v: three DoubleRow chains (hi*hi + lo*hi + hi*lo)
    accumulate in ONE x32 PSUM group = ~bf16 accuracy at 75% of the bf16
    matmul cost; RoPE reads the PSUM directly (rotate-half crosses
    partitions, legal only for PSUM sources); v is computed directly in
    [tk, dh] layout (lhsT = x) so no transpose is needed
  - attention: transposed-score layout S^T[tk,tq], scores bf16, causal
    mask as a post-exp 0/1 multiply on Pool; strip j=0 keeps bf16 probs;
    strips 1-3 exp to fp8 with a -4.5 offset (cancels in the deferred
    softmax normalization) and run PV + rowsum as fp8 DoubleRow over
    tk-tile pairs (the rowsum ones-vector is 64 wide - dual-fp8
    Ldweights rejects narrow weight tiles)
  - wo: fp8 3-chain DoubleRow with heads as the pair dim; outT splits to
    hi + lo on device; partials staged bf16 at x512, host sums in f32
  - schedule: per-strip software pipeline (scores+exps phase, then PV
    phase with a full strip of slack), wo tiles and deferred epilogues
    ride as fillers inside the ACT-paced score streams, PE warmup
    matmuls bridge the initial DMA wait to keep the p-state ramp hot
"""

import math
import os
import sys
import time

for _p in ("/opt/trn_rl_repo", "/root/.axon_site/_ro/trn_rl_repo"):
    if os.path.isdir(_p) and _p not in sys.path:
        sys.path.insert(0, _p)

import numpy as np
import ml_dtypes

import concourse.bass as bass
import concourse.tile as tile
from concourse import bacc, mybir
from concourse.bass_utils import run_bass_kernel_spmd

BF16 = mybir.dt.bfloat16
F8 = mybir.dt.float8e4
F32R = mybir.dt.float32r
F32 = mybir.dt.float32
AF = mybir.ActivationFunctionType
ALU = mybir.AluOpType
DR = mybir.MatmulPerfMode.DoubleRow

T = 2048
D = 2048
N_H = 16
D_H = 128
N_CORES = 8
H_LOC = N_H // N_CORES          # heads per core = 2
NL = H_LOC * D_H                # local head width = 256
KD = D // 128                   # contraction tiles = 16
KP = KD // 2                    # DoubleRow contraction pairs = 8
TT = T // 128                   # t tiles = 16
NS = T // 512                   # 512-wide strips = 4
EPS = 1e-5
INV_SQRT_DH = 1.0 / math.sqrt(D_H)
EOFF = 4.5                      # fp8 exp offset (cancels in normalization)
WSC = 32.0                      # fp8 weight scale (hi and lo residual alike)
OSC = 512.0                     # output scale (16 from v, 32 from wo weights)

_CACHED = {}
_MARKS = []          # (label, next-instruction-name) pairs when _MARK_ON
_MARK_ON = [False]   # analysis hook; stays off in normal runs


def _build_program(repeats=1):
    if repeats in _CACHED:
        return _CACHED[repeats]

    nc = bacc.Bacc("TRN2", target_bir_lowering=False, debug=False, num_devices=N_CORES)

    # all bulk inputs host-prepacked to partition-major [128, ...] so every
    # DMA is 128 long contiguous runs
    xhi_ds = [nc.dram_tensor(f"xhi{j}", [128, KD * 512], F8, kind="ExternalInput")
              for j in range(NS)]
    xlo_ds = [nc.dram_tensor(f"xlo{j}", [128, KD * 512], F8, kind="ExternalInput")
              for j in range(NS)]
    wq_hi_d = nc.dram_tensor("wqhi", [128, KD * NL], F8, kind="ExternalInput")
    wq_lo_d = nc.dram_tensor("wqlo", [128, KD * NL], F8, kind="ExternalInput")
    wk_hi_d = nc.dram_tensor("wkhi", [128, KD * NL], F8, kind="ExternalInput")
    wk_lo_d = nc.dram_tensor("wklo", [128, KD * NL], F8, kind="ExternalInput")
    wv_hi_d = nc.dram_tensor("wvhi", [128, KD * NL], F8, kind="ExternalInput")
    wv_lo_d = nc.dram_tensor("wvlo", [128, KD * NL], F8, kind="ExternalInput")
    wo_hi_d = nc.dram_tensor("wohi", [128, H_LOC * T], F8, kind="ExternalInput")
    wo_lo_d = nc.dram_tensor("wolo", [128, H_LOC * T], F8, kind="ExternalInput")
    cos_d = nc.dram_tensor("cosT", [D_H, T], BF16, kind="ExternalInput")
    sin_d = nc.dram_tensor("sinT", [D_H, T], BF16, kind="ExternalInput")
    skt_d = nc.dram_tensor("skt", [128, TT], F32, kind="ExternalInput")
    mskb_d = nc.dram_tensor("maskb", [128, 128], BF16, kind="ExternalInput")
    onc_d = nc.dram_tensor("ones_col", [1, 128], F32R, kind="ExternalInput")
    on128_d = nc.dram_tensor("ones128", [128, 1], BF16, kind="ExternalInput")
    on2_d = nc.dram_tensor("ones2f8", [128, 128], F8, kind="ExternalInput")
    out_d = nc.dram_tensor("out", [T, D], BF16, kind="ExternalOutput")

    ap = lambda h: h.ap()
    out_ap = ap(out_d)

    from contextlib import ExitStack

    with tile.TileContext(nc) as tc, ExitStack() as ctx:
        P = ctx.enter_context  # noqa

        singles = P(tc.tile_pool(name="singles", bufs=1))
        rope = P(tc.tile_pool(name="rope", bufs=4))        # [128,512] bf16
        epool = P(tc.tile_pool(name="epool", bufs=10))     # [128,2,512] f8 pairs
        e0pool = P(tc.tile_pool(name="e0pool", bufs=8))    # [128,512] bf16 strip0
        rbsp = P(tc.tile_pool(name="rbsp", bufs=2))        # [128,512] bf16 pv evac
        small = P(tc.tile_pool(name="small", bufs=2))      # [1,512] f32
        stage = P(tc.tile_pool(name="stage", bufs=4))      # [128,T] bf16 out staging
        pA = P(tc.tile_pool(name="pA", bufs=3, space="PSUM"))   # proj + wo
        psc = P(tc.tile_pool(name="psc", bufs=3, space="PSUM")) # scores + tail wo
        ppv = P(tc.tile_pool(name="ppv", bufs=1, space="PSUM")) # po + rb
        psu = P(tc.tile_pool(name="psu", bufs=1, space="PSUM")) # su

        def mark(label):
            if _MARK_ON[0]:
                _MARKS.append((label, nc.get_next_instruction_name()))

        def emit_body():
            mark("loads")
            # ---------------- phase 0: loads in consumption order ------------------
            # PE warmup: ~5us of tiny matmuls during the initial DMA wait so the
            # p-state ramp completes before real work arrives
            warm = singles.tile([128, 256], BF16, tag="warm")
            nc.gpsimd.memset(warm, 1.0)
            eoffb = singles.tile([128, 1], F32, tag="eoffb")
            nc.vector.memset(eoffb, -EOFF)
            for _w in range(16):
                pw = psc.tile([128, 256], F32, tag="sc", name=f"warm{_w}")
                nc.tensor.matmul(pw, lhsT=warm[:, 0:128], rhs=warm,
                                 start=True, stop=True)

            xhi_t = [
                singles.tile([128, KD, 512], F8, tag=f"xhi{j}", name=f"xhi{j}")
                for j in range(NS)
            ]
            xlo_t = [
                singles.tile([128, KD, 512], F8, tag=f"xlo{j}", name=f"xlo{j}")
                for j in range(NS)
            ]

            def load_x_strip(j, hi, half=None):
                t_, d_ = (xhi_t[j], xhi_ds[j]) if hi else (xlo_t[j], xlo_ds[j])
                dv = ap(d_).rearrange("p (a m) -> p a m", a=KD)
                if half is None:
                    nc.sync.dma_start(out=t_, in_=dv)
                elif half == 0:
                    nc.sync.dma_start(out=t_[:, 0:8, :], in_=dv[:, 0:8, :])
                else:
                    nc.sync.dma_start(out=t_[:, 8:16, :], in_=dv[:, 8:16, :])

            def load_w(dram, tag, split=False):
                t_ = singles.tile([128, KD, NL], F8, tag=tag)
                dv = ap(dram).rearrange("p (a m) -> p a m", a=KD)
                if split:
                    nc.sync.dma_start(out=t_[:, 0:8, :], in_=dv[:, 0:8, :])
                    nc.sync.dma_start(out=t_[:, 8:16, :], in_=dv[:, 8:16, :])
                else:
                    nc.sync.dma_start(out=t_, in_=dv)
                return t_

            # interleave x0/w halves so the first chains start ~3us in
            wkh = singles.tile([128, KD, NL], F8, tag="wkh")
            wkhv = ap(wk_hi_d).rearrange("p (a m) -> p a m", a=KD)
            nc.sync.dma_start(out=wkh[:, 0:8, :], in_=wkhv[:, 0:8, :])
            load_x_strip(0, True, 0)
            nc.sync.dma_start(out=wkh[:, 8:16, :], in_=wkhv[:, 8:16, :])
            load_x_strip(0, True, 1)
            wkl = load_w(wk_lo_d, "wkl", split=True)
            load_x_strip(0, False, 0)
            load_x_strip(0, False, 1)
            wqh = load_w(wq_hi_d, "wqh", split=True)
            wql = load_w(wq_lo_d, "wql", split=True)
            cos_s = singles.tile([128, T], BF16, tag="cos_s")
            nc.sync.dma_start(out=cos_s, in_=ap(cos_d))
            sin_s = singles.tile([128, T], BF16, tag="sin_s")
            nc.sync.dma_start(out=sin_s, in_=ap(sin_d))
            wvh = load_w(wv_hi_d, "wvh")
            wvl = load_w(wv_lo_d, "wvl")
            sk_t = singles.tile([128, TT], F32, tag="sk")
            nc.sync.dma_start(out=sk_t, in_=ap(skt_d))
            for j in range(1, NS):
                load_x_strip(j, True)
                load_x_strip(j, False)
            del wkhv
            mskb = singles.tile([128, 128], BF16, tag="mskb")
            nc.sync.dma_start(out=mskb, in_=ap(mskb_d))
            onc = singles.tile([1, 128], F32R, tag="onc")
            nc.sync.dma_start(out=onc, in_=ap(onc_d))
            on128 = singles.tile([128, 1], BF16, tag="on128")
            nc.sync.dma_start(out=on128, in_=ap(on128_d))
            on2 = singles.tile([128, 2, 64], F8, tag="on2")
            nc.sync.dma_start(out=on2, in_=ap(on2_d).rearrange("p (a b) -> p a b", a=2))
            woh = singles.tile([128, H_LOC, T], F8, tag="woh")
            nc.sync.dma_start(out=woh, in_=ap(wo_hi_d).rearrange("p (h t) -> p h t", h=H_LOC))
            wol = singles.tile([128, H_LOC, T], F8, tag="wol")
            nc.sync.dma_start(out=wol, in_=ap(wo_lo_d).rearrange("p (h t) -> p h t", h=H_LOC))

            # ---------------- projections ------------------------------------------
            q_sb = singles.tile([128, H_LOC, T], BF16, tag="q_sb")
            k_sb = singles.tile([128, H_LOC, T], BF16, tag="k_sb")
            v8 = singles.tile([128, TT, NL], F8, tag="v8")
            v0 = singles.tile([128, 4, NL], BF16, tag="v0")
            outT = singles.tile([128, H_LOC, T], BF16, tag="outT")
            ohi = singles.tile([128, H_LOC, T], F8, tag="ohi")
            olo = singles.tile([128, H_LOC, T], F8, tag="olo")

            def emit_proj_psum(whi, wlo, h, j):
                """Single-group DoubleRow projection at x32: hi*hi + hi*lo +
                lo*hi (residuals stored at the same scale as hi)."""
                hs = slice(h * 128, (h + 1) * 128)
                xh, xl = xhi_t[j], xlo_t[j]
                ps = pA.tile([128, 512], F32, tag="pa")
                for kp in range(KP):
                    ks = slice(2 * kp, 2 * kp + 2)
                    nc.tensor.matmul(
                        ps, lhsT=whi[:, ks, hs], rhs=xh[:, ks, :],
                        start=(kp == 0), stop=False, perf_mode=DR,
                    )
                for kp in range(KP):
                    ks = slice(2 * kp, 2 * kp + 2)
                    nc.tensor.matmul(
                        ps, lhsT=wlo[:, ks, hs], rhs=xh[:, ks, :],
                        start=False, stop=False, perf_mode=DR,
                    )
                for kp in range(KP):
                    ks = slice(2 * kp, 2 * kp + 2)
                    nc.tensor.matmul(
                        ps, lhsT=whi[:, ks, hs], rhs=xl[:, ks, :],
                        start=False, stop=(kp == KP - 1), perf_mode=DR,
                    )
                return ps

            def emit_qk_strip(h, j):
                mark(f"qk h{h} j{j}")
                js = slice(j * 512, (j + 1) * 512)
                for dst, whi, wlo in ((k_sb, wkh, wkl), (q_sb, wqh, wql)):
                    ps = emit_proj_psum(whi, wlo, h, j)
                    # RoPE: rotate-half via a partition-swapping DMA (engines
                    # cannot cross partitions); tables carry s/32 and the sign
                    m1 = rope.tile([128, 512], BF16, tag="m1")
                    nc.vector.tensor_mul(m1, ps, cos_s[:, js])
                    m2 = rope.tile([128, 512], BF16, tag="m2")
                    nc.vector.tensor_mul(m2[0:64, :], ps[64:128, :], sin_s[0:64, js])
                    nc.vector.tensor_mul(m2[64:128, :], ps[0:64, :], sin_s[64:128, js])
                    nc.vector.tensor_add(dst[:, h, js], m1, m2)

            def emit_v_tile(tt):
                # v in [tk, dh] layout directly: lhsT = x (tk columns as the
                # stationary free dim), rhs = wv; no transpose needed.
                mark(f"v tt{tt}")
                j, lt = tt // 4, tt % 4
                ls = slice(lt * 128, (lt + 1) * 128)
                xh, xl = xhi_t[j], xlo_t[j]
                ps = pA.tile([128, NL], F32, tag="pa", name="vps")
                for kp in range(KP):
                    ks = slice(2 * kp, 2 * kp + 2)
                    nc.tensor.matmul(
                        ps, lhsT=xh[:, ks, ls], rhs=wvh[:, ks, :],
                        start=(kp == 0), stop=False, perf_mode=DR,
                    )
                for kp in range(KP):
                    ks = slice(2 * kp, 2 * kp + 2)
                    nc.tensor.matmul(
                        ps, lhsT=xl[:, ks, ls], rhs=wvh[:, ks, :],
                        start=False, stop=False, perf_mode=DR,
                    )
                for kp in range(KP):
                    ks = slice(2 * kp, 2 * kp + 2)
                    nc.tensor.matmul(
                        ps, lhsT=xh[:, ks, ls], rhs=wvl[:, ks, :],
                        start=False, stop=(kp == KP - 1), perf_mode=DR,
                    )
                nc.scalar.activation(v8[:, tt, :], ps, AF.Copy,
                                     scale=sk_t[:, tt : tt + 1])
                if tt < 4:
                    nc.vector.tensor_scalar_mul(v0[:, tt, :], ps, sk_t[:, tt : tt + 1])

            # ---------------- attention --------------------------------------------
            # A phase: all scores + exps of a strip (PE streams scores while
            # Pool/ACT chase with mask + exp). B phase: PV + rowsum + epilogue
            # (runs with a full strip of slack behind the exps).
            e_units = {}
            fin = {}

            def emit_att_A(h, j, fillers=()):
                mark(f"attA h{h} j{j}")
                fillers = list(fillers)
                nunit = 0

                def tick():
                    nonlocal nunit
                    nunit += 1
                    if nunit % 2 == 1 and fillers:
                        fillers.pop(0)()

                js = slice(j * 512, (j + 1) * 512)
                units = []
                if j == 0:
                    for i in range(4):
                        c0 = 128 * i
                        cs = slice(c0, 512)
                        st = psc.tile([128, 512], F32, tag="sc")
                        nc.tensor.matmul(
                            st[:, cs], lhsT=k_sb[:, h, i * 128 : (i + 1) * 128],
                            rhs=q_sb[:, h, cs], start=True, stop=True,
                        )
                        e0 = e0pool.tile([128, 512], BF16, tag="e0")
                        nc.scalar.activation(e0[:, cs], st[:, cs], AF.Exp,
                                             scale=INV_SQRT_DH)
                        nc.gpsimd.tensor_mul(
                            e0[:, c0 : c0 + 128], e0[:, c0 : c0 + 128], mskb
                        )
                        units.append((e0, c0))
                        tick()
                else:
                    npair = 2 * (j + 1)
                    for p_ in range(npair):
                        i0 = 2 * p_
                        r0 = i0 - 4 * j
                        c0p = 128 * r0 if r0 >= 0 else 0
                        e8 = epool.tile([128, 2, 512], F8, tag="e8")
                        for slot in range(2):
                            i = i0 + slot
                            r = i - 4 * j
                            c0 = 128 * r if r > 0 else 0
                            cs = slice(c0, 512)
                            qs = slice(j * 512 + c0, (j + 1) * 512)
                            st = psc.tile([128, 512], F32, tag="sc")
                            nc.tensor.matmul(
                                st[:, cs], lhsT=k_sb[:, h, i * 128 : (i + 1) * 128],
                                rhs=q_sb[:, h, qs], start=True, stop=True,
                            )
                            nc.scalar.activation(e8[:, slot, cs], st[:, cs], AF.Exp,
                                                 scale=INV_SQRT_DH, bias=eoffb)
                            if r >= 0:
                                nc.gpsimd.tensor_mul(
                                    e8[:, slot, c0 : c0 + 128],
                                    e8[:, slot, c0 : c0 + 128], mskb
                                )
                            if slot == 1 and c0 > c0p:
                                nc.gpsimd.memset(e8[:, 1, c0p:c0], 0.0)
                        units.append((e8, c0p))
                        tick()
                e_units[(h, j)] = units
                for f in fillers:
                    f()

            def emit_att_B(h, j):
                mark(f"attB h{h} j{j}")
                hs = slice(h * 128, (h + 1) * 128)
                js = slice(j * 512, (j + 1) * 512)
                units = e_units.pop((h, j))
                po = ppv.tile([128, 512], F32, tag="pv")
                su = psu.tile([64, 512], F32, tag="su")
                n = len(units)
                if j == 0:
                    for i, (e0, c0) in enumerate(units):
                        cs = slice(c0, 512)
                        nc.tensor.matmul(
                            po[:, cs], lhsT=v0[:, i, hs], rhs=e0[:, cs],
                            start=(i == 0), stop=(i == n - 1),
                        )
                        nc.tensor.matmul(
                            su[0:1, cs], lhsT=on128, rhs=e0[:, cs],
                            start=(i == 0), stop=(i == n - 1),
                        )
                else:
                    for p_, (e8, c0p) in enumerate(units):
                        csp = slice(c0p, 512)
                        i0 = 2 * p_
                        nc.tensor.matmul(
                            po[:, csp], lhsT=v8[:, i0 : i0 + 2, hs],
                            rhs=e8[:, :, csp],
                            start=(p_ == 0), stop=(p_ == n - 1), perf_mode=DR,
                        )
                        nc.tensor.matmul(
                            su[:, csp], lhsT=on2, rhs=e8[:, :, csp],
                            start=(p_ == 0), stop=(p_ == n - 1), perf_mode=DR,
                        )  # 64 identical rowsum rows; row 0 is used
                pos = rbsp.tile([128, 512], BF16, tag="pos")
                nc.vector.tensor_copy(pos, po)
                rec = small.tile([1, 512], F32R, tag="rec")
                with nc.allow_low_precision(reason="1/su feeds an f32r bcast"):
                    nc.vector.reciprocal(rec, su[0:1, :])
                fin[(h, j)] = (pos, rec)

            def emit_att_B2(h, j):
                mark(f"attB2 h{h} j{j}")
                js = slice(j * 512, (j + 1) * 512)
                pos, rec = fin.pop((h, j))
                rb = ppv.tile([128, 512], F32, tag="pv", name="rb")
                nc.tensor.matmul(rb, lhsT=onc, rhs=rec, start=True, stop=True)
                nc.vector.tensor_mul(outT[:, h, js], rb, pos)
                nc.gpsimd.tensor_copy(ohi[:, h, js], outT[:, h, js])
                nc.gpsimd.tensor_sub(olo[:, h, js], outT[:, h, js], ohi[:, h, js])

            def emit_wo_tile(tt, tail=False):
                # fp8 3-chain in one x512 PSUM group: (ohi+olo)*wohi +
                # (outT/16 as fp8)*wolo; host divides by 512
                mark(f"wo tt{tt}")
                ts = slice(tt * 128, (tt + 1) * 128)
                stg = stage.tile([128, T], BF16, tag="stg")
                for n in range(NS):
                    ns = slice(n * 512, (n + 1) * 512)
                    if tail and n % 2 == 1:
                        pa = psc.tile([128, 512], F32, tag="sc", name="wot")
                    else:
                        pa = pA.tile([128, 512], F32, tag="pa")
                    nc.tensor.matmul(
                        pa, lhsT=ohi[:, :, ts], rhs=woh[:, :, ns],
                        start=True, stop=False, perf_mode=DR,
                    )
                    nc.tensor.matmul(
                        pa, lhsT=olo[:, :, ts], rhs=woh[:, :, ns],
                        start=False, stop=False, perf_mode=DR,
                    )
                    nc.tensor.matmul(
                        pa, lhsT=ohi[:, :, ts], rhs=wol[:, :, ns],
                        start=False, stop=True, perf_mode=DR,
                    )
                    if n % 2 == 0:
                        nc.vector.tensor_copy(stg[:, ns], pa)
                    else:
                        nc.scalar.copy(stg[:, ns], pa)
                    if n == 1:
                        nc.sync.dma_start(out=out_ap[ts, 0:1024], in_=stg[:, 0:1024])
                nc.sync.dma_start(out=out_ap[ts, 1024:2048], in_=stg[:, 1024:2048])

            # emission order mirrors data readiness: attention strip j only
            # needs x strips <= j, so each strip's qk/v/attention pipeline
            # rides directly behind its x DMA; wo tiles and deferred
            # epilogues (B2) fill the ACT-paced score streams.
            wo_f = lambda tt: (lambda: emit_wo_tile(tt))
            b2_f = lambda h, j: (lambda: emit_att_B2(h, j))

            emit_qk_strip(0, 0)
            emit_qk_strip(1, 0)
            for tt in range(0, 4):
                emit_v_tile(tt)
            emit_att_A(0, 0)
            emit_att_B(0, 0)
            emit_att_A(1, 0, [b2_f(0, 0)])
            emit_att_B(1, 0)
            emit_qk_strip(0, 1)
            emit_qk_strip(1, 1)
            for tt in range(4, 8):
                emit_v_tile(tt)
            emit_att_A(0, 1, [b2_f(1, 0)])
            emit_att_B(0, 1)
            emit_att_A(1, 1, [b2_f(0, 1)])
            emit_att_B(1, 1)
            emit_qk_strip(0, 2)
            emit_qk_strip(1, 2)
            for tt in range(8, 12):
                emit_v_tile(tt)
            emit_att_A(0, 2, [b2_f(1, 1), wo_f(0), wo_f(1)])
            emit_att_B(0, 2)
            emit_att_A(1, 2, [b2_f(0, 2), wo_f(2), wo_f(3)])
            emit_att_B(1, 2)
            emit_qk_strip(0, 3)
            emit_qk_strip(1, 3)
            for tt in range(12, 16):
                emit_v_tile(tt)
            emit_att_A(0, 3, [b2_f(1, 2), wo_f(4), wo_f(5), wo_f(6)])
            emit_att_B(0, 3)
            emit_att_A(1, 3, [b2_f(0, 3), wo_f(7), wo_f(8), wo_f(9), wo_f(10),
                              wo_f(11)])
            emit_att_B(1, 3)
            emit_att_B2(1, 3)
            for tt in (12, 13, 14, 15):
                emit_wo_tile(tt, tail=True)

        for _rep in range(repeats):
            emit_body()

    # Force Exp and Ln onto the single combined table set (one
    # ACT_TABLE_LOAD for the whole kernel).
    from concourse.hw_specs import get_activation_tables
    tabs = get_activation_tables(nc.m.arch)
    for nm_, fs_ in tabs.items():
        if nm_ != "natural_log_exp_and_others":
            fs_.discard(AF.Exp)
            fs_.discard(AF.Ln)
    nc.compile()
    _CACHED[repeats] = nc
    return nc


def _host_prep(x, w_ln, wq, wk, wv, wo, cos, sin):
    bf = ml_dtypes.bfloat16
    f8 = ml_dtypes.float8_e4m3
    x = np.asarray(x, np.float32)
    w_ln = np.asarray(w_ln, np.float32)

    # per-token RMSNorm scale, folded into the RoPE tables and v scale
    s = 1.0 / np.sqrt((x * x).mean(axis=1) + EPS)          # [T] f32

    cosT = np.ascontiguousarray(np.asarray(cos, np.float32).T)   # [D_H, T]
    sinT = np.ascontiguousarray(np.asarray(sin, np.float32).T)
    sinT[0:64] *= -1.0          # rotate_half sign folded into the table
    cos_s = (cosT * (s / WSC)[None, :]).astype(bf)
    sin_s = (sinT * (s / WSC)[None, :]).astype(bf)
    # v8 carries x16 (for the fp8 wo split); proj psum carries x32
    skt = np.ascontiguousarray((s * 16.0 / WSC).reshape(TT, 128).T).astype(np.float32)

    xT = np.ascontiguousarray(x.T).astype(bf).astype(np.float32)
    x_hi = xT.astype(f8)
    x_lo = (xT - x_hi.astype(np.float32)).astype(f8)     # same scale as hi

    def pack_kd(a, ncols):
        # [D, M] -> [128, KD*M]: tile[p, kd*M + m] = a[kd*128 + p, m]
        return np.ascontiguousarray(
            a.reshape(KD, 128, ncols).transpose(1, 0, 2).reshape(128, KD * ncols))

    xhi_p = {f"xhi{j}": pack_kd(x_hi[:, j * 512:(j + 1) * 512], 512) for j in range(NS)}
    xlo_p = {f"xlo{j}": pack_kd(x_lo[:, j * 512:(j + 1) * 512], 512) for j in range(NS)}

    # causal boundary: 1 where tq >= tk within the tile, else 0
    f = np.arange(128)[None, :]
    p = np.arange(128)[:, None]
    maskb = (f >= p).astype(bf)

    ones_col = np.ones((1, 128), np.float32)
    ones128 = np.ones((128, 1), bf)
    ones2f8 = np.ones((128, 128), f8)

    wq_s = np.asarray(wq, np.float32) * w_ln[None, :]
    wk_s = np.asarray(wk, np.float32) * w_ln[None, :]
    wv_s = np.asarray(wv, np.float32) * w_ln[None, :]
    wo32 = np.asarray(wo, np.float32)

    def wsplit(w_sl):
        # [D, NL] slice, bf16-rounded like the reference weights path;
        # hi and the residual lo are stored at the same x32 scale
        wT = np.ascontiguousarray(w_sl.T).astype(bf).astype(np.float32) * WSC
        hi = wT.astype(f8)
        lo = (wT - hi.astype(np.float32)).astype(f8)
        return pack_kd(hi, NL), pack_kd(lo, NL)

    in_maps = []
    for c in range(N_CORES):
        sl = slice(c * NL, (c + 1) * NL)
        wqh, wql = wsplit(wq_s[sl])
        wkh, wkl = wsplit(wk_s[sl])
        wvh, wvl = wsplit(wv_s[sl])
        woT = np.ascontiguousarray(wo32[:, sl].T).astype(bf).astype(np.float32) * WSC
        woT_hi = woT.astype(f8)
        woT_lo = (woT - woT_hi.astype(np.float32)).astype(f8)

        def pack_h(a):
            return np.ascontiguousarray(
                a.reshape(H_LOC, 128, T).transpose(1, 0, 2).reshape(128, H_LOC * T))

        in_maps.append({
            **xhi_p, **xlo_p,
            "wqhi": wqh, "wqlo": wql,
            "wkhi": wkh, "wklo": wkl,
            "wvhi": wvh, "wvlo": wvl,
            "wohi": pack_h(woT_hi), "wolo": pack_h(woT_lo),
            "cosT": cos_s,
            "sinT": sin_s,
            "skt": skt,
            "maskb": maskb,
            "ones_col": ones_col,
            "ones128": ones128,
            "ones2f8": ones2f8,
        })
    return in_maps


def kernel(x, w_ln, wq, wk, wv, wo, cos, sin):
    nc = _build_program()
    in_maps = _host_prep(x, w_ln, wq, wk, wv, wo, cos, sin)
    t0 = time.time()
    res = run_bass_kernel_spmd(nc, in_maps, core_ids=list(range(N_CORES)))
    t1 = time.time()
    print(f"run_bass_kernel_spmd wall: {(t1 - t0) * 1e3:.1f} ms", file=sys.stderr)
    acc = np.zeros((T, D), np.float32)
    for r in res.results:
        acc += np.asarray(r["out"], np.float32)
    return np.asarray(x, np.float32) + acc * (1.0 / OSC)


# revision 57
# speedup vs baseline: 768.9292x; 1.0157x over previous
"""Trainium2 Bass kernel for nn_Attention (T=2048, D=2048, H=16, Dh=128).

Strategy: tensor-parallel over heads, 2 heads per core on 8 cores.
fp8 (e4m3) DoubleRow matmuls everywhere precision allows, with same-scale
residual compensation so the result keeps bf16-level accuracy:

  - host prep: w_ln and the per-token RMSNorm scale s fold into the RoPE
    tables / v scale; every operand is split into fp8 hi + residual lo at
    the SAME scale (w at x32, x at x1) and prepacked partition-major so
    all DMAs are 128 long contiguous runs
  - projections q/k/v: three DoubleRow chains (hi*hi + lo*hi + hi*lo)
    accumulate in ONE x32 PSUM group = ~bf16 accuracy at 75% of the bf16
    matmul cost; RoPE reads the PSUM directly (rotate-half crosses
    partitions, legal only for PSUM sources); v is computed directly in
    [tk, dh] layout (lhsT = x) so no transpose is needed
  - attention: transposed-score layout S^T[tk,tq], scores bf16, causal
    mask as a post-exp 0/1 multiply on Pool; strip j=0 keeps bf16 probs;
    strips 1-3 exp to fp8 with a -4.5 offset (cancels in the deferred
    softmax normalization) and run PV + rowsum as fp8 DoubleRow over
    tk-tile pairs (the rowsum ones-vector is 64 wide - dual-fp8
    Ldweights rejects narrow weight tiles)
  - wo: fp8 3-chain DoubleRow with heads as the pair dim; outT splits to
    hi + lo on device; partials staged bf16 at x512, host sums in f32
  - schedule: per-strip software pipeline (scores+exps phase, then PV
    phase with a full strip of slack), wo tiles and deferred epilogues
    ride as fillers inside the ACT-paced score streams, PE warmup
    matmuls bridge the initial DMA wait to keep the p-state ramp hot
"""

import math
import os
import sys
import time

for _p in ("/opt/trn_rl_repo", "/root/.axon_site/_ro/trn_rl_repo"):
    if os.path.isdir(_p) and _p not in sys.path:
        sys.path.insert(0, _p)

import numpy as np
import ml_dtypes

import concourse.bass as bass
import concourse.tile as tile
from concourse import bacc, mybir
from concourse.bass_utils import run_bass_kernel_spmd

BF16 = mybir.dt.bfloat16
F8 = mybir.dt.float8e4
F32R = mybir.dt.float32r
F32 = mybir.dt.float32
AF = mybir.ActivationFunctionType
ALU = mybir.AluOpType
DR = mybir.MatmulPerfMode.DoubleRow

T = 2048
D = 2048
N_H = 16
D_H = 128
N_CORES = 8
H_LOC = N_H // N_CORES          # heads per core = 2
NL = H_LOC * D_H                # local head width = 256
KD = D // 128                   # contraction tiles = 16
KP = KD // 2                    # DoubleRow contraction pairs = 8
TT = T // 128                   # t tiles = 16
NS = T // 512                   # 512-wide strips = 4
EPS = 1e-5
INV_SQRT_DH = 1.0 / math.sqrt(D_H)
EOFF = 4.5                      # fp8 exp offset (cancels in normalization)
WSC = 32.0                      # fp8 weight scale (hi and lo residual alike)
OSC = 512.0                     # output scale (16 from v, 32 from wo weights)

_CACHED = {}
_MARKS = []          # (label, next-instruction-name) pairs when _MARK_ON
_MARK_ON = [False]   # analysis hook; stays off in normal runs


def _build_program(repeats=1):
    if repeats in _CACHED:
        return _CACHED[repeats]

    nc = bacc.Bacc("TRN2", target_bir_lowering=False, debug=False, num_devices=N_CORES)

    # all bulk inputs host-prepacked to partition-major [128, ...] so every
    # DMA is 128 long contiguous runs
    xhi_ds = [nc.dram_tensor(f"xhi{j}", [128, KD * 512], F8, kind="ExternalInput")
              for j in range(NS)]
    xlo_ds = [nc.dram_tensor(f"xlo{j}", [128, KD * 512], F8, kind="ExternalInput")
              for j in range(NS)]
    wq_hi_d = nc.dram_tensor("wqhi", [128, KD * NL], F8, kind="ExternalInput")
    wq_lo_d = nc.dram_tensor("wqlo", [128, KD * NL], F8, kind="ExternalInput")
    wk_hi_d = nc.dram_tensor("wkhi", [128, KD * NL], F8, kind="ExternalInput")
    wk_lo_d = nc.dram_tensor("wklo", [128, KD * NL], F8, kind="ExternalInput")
    wv_hi_d = nc.dram_tensor("wvhi", [128, KD * NL], F8, kind="ExternalInput")
    wv_lo_d = nc.dram_tensor("wvlo", [128, KD * NL], F8, kind="ExternalInput")
    wo_hi_d = nc.dram_tensor("wohi", [128, H_LOC * T], F8, kind="ExternalInput")
    wo_lo_d = nc.dram_tensor("wolo", [128, H_LOC * T], F8, kind="ExternalInput")
    cos_d = nc.dram_tensor("cosT", [D_H, T], BF16, kind="ExternalInput")
    sin_d = nc.dram_tensor("sinT", [D_H, T], BF16, kind="ExternalInput")
    skt_d = nc.dram_tensor("skt", [128, TT], F32, kind="ExternalInput")
    mskb_d = nc.dram_tensor("maskb", [128, 128], BF16, kind="ExternalInput")
    onc_d = nc.dram_tensor("ones_col", [1, 128], F32R, kind="ExternalInput")
    on128_d = nc.dram_tensor("ones128", [128, 1], BF16, kind="ExternalInput")
    on2_d = nc.dram_tensor("ones2f8", [128, 128], F8, kind="ExternalInput")
    out_d = nc.dram_tensor("out", [T, D], BF16, kind="ExternalOutput")

    ap = lambda h: h.ap()
    out_ap = ap(out_d)

    from contextlib import ExitStack

    with tile.TileContext(nc) as tc, ExitStack() as ctx:
        P = ctx.enter_context  # noqa

        singles = P(tc.tile_pool(name="singles", bufs=1))
        rope = P(tc.tile_pool(name="rope", bufs=4))        # [128,512] bf16
        epool = P(tc.tile_pool(name="epool", bufs=10))     # [128,2,512] f8 pairs
        e0pool = P(tc.tile_pool(name="e0pool", bufs=8))    # [128,512] bf16 strip0
        rbsp = P(tc.tile_pool(name="rbsp", bufs=2))        # [128,512] bf16 pv evac
        small = P(tc.tile_pool(name="small", bufs=2))      # [1,512] f32
        stage = P(tc.tile_pool(name="stage", bufs=4))      # [128,T] bf16 out staging
        pA = P(tc.tile_pool(name="pA", bufs=3, space="PSUM"))   # proj + wo
        psc = P(tc.tile_pool(name="psc", bufs=3, space="PSUM")) # scores + tail wo
        ppv = P(tc.tile_pool(name="ppv", bufs=1, space="PSUM")) # po + rb
        psu = P(tc.tile_pool(name="psu", bufs=1, space="PSUM")) # su

        def mark(label):
            if _MARK_ON[0]:
                _MARKS.append((label, nc.get_next_instruction_name()))

        def emit_body():
            mark("loads")
            # ---------------- phase 0: loads in consumption order ------------------
            # PE warmup: ~5us of tiny matmuls during the initial DMA wait so the
            # p-state ramp completes before real work arrives
            warm = singles.tile([128, 256], BF16, tag="warm")
            nc.gpsimd.memset(warm, 1.0)
            eoffb = singles.tile([128, 1], F32, tag="eoffb")
            nc.vector.memset(eoffb, -EOFF)
            for _w in range(16):
                pw = psc.tile([128, 256], F32, tag="sc", name=f"warm{_w}")
                nc.tensor.matmul(pw, lhsT=warm[:, 0:128], rhs=warm,
                                 start=True, stop=True)

            xhi_t = [
                singles.tile([128, KD, 512], F8, tag=f"xhi{j}", name=f"xhi{j}")
                for j in range(NS)
            ]
            xlo_t = [
                singles.tile([128, KD, 512], F8, tag=f"xlo{j}", name=f"xlo{j}")
                for j in range(NS)
            ]

            def load_x_strip(j, hi, half=None):
                t_, d_ = (xhi_t[j], xhi_ds[j]) if hi else (xlo_t[j], xlo_ds[j])
                dv = ap(d_).rearrange("p (a m) -> p a m", a=KD)
                if half is None:
                    nc.sync.dma_start(out=t_, in_=dv)
                elif half == 0:
                    nc.sync.dma_start(out=t_[:, 0:8, :], in_=dv[:, 0:8, :])
                else:
                    nc.sync.dma_start(out=t_[:, 8:16, :], in_=dv[:, 8:16, :])

            def load_w(dram, tag, split=False):
                t_ = singles.tile([128, KD, NL], F8, tag=tag)
                dv = ap(dram).rearrange("p (a m) -> p a m", a=KD)
                if split:
                    nc.sync.dma_start(out=t_[:, 0:8, :], in_=dv[:, 0:8, :])
                    nc.sync.dma_start(out=t_[:, 8:16, :], in_=dv[:, 8:16, :])
                else:
                    nc.sync.dma_start(out=t_, in_=dv)
                return t_

            # interleave x0/w halves so the first chains start ~3us in
            wkh = singles.tile([128, KD, NL], F8, tag="wkh")
            wkhv = ap(wk_hi_d).rearrange("p (a m) -> p a m", a=KD)
            nc.sync.dma_start(out=wkh[:, 0:8, :], in_=wkhv[:, 0:8, :])
            load_x_strip(0, True, 0)
            nc.sync.dma_start(out=wkh[:, 8:16, :], in_=wkhv[:, 8:16, :])
            load_x_strip(0, True, 1)
            wkl = load_w(wk_lo_d, "wkl", split=True)
            load_x_strip(0, False, 0)
            load_x_strip(0, False, 1)
            wqh = load_w(wq_hi_d, "wqh", split=True)
            wql = load_w(wq_lo_d, "wql", split=True)
            # strip-0 table columns first: RoPE j0 unblocks before the wv bulk
            cos_s = singles.tile([128, T], BF16, tag="cos_s")
            nc.sync.dma_start(out=cos_s[:, 0:512], in_=ap(cos_d)[:, 0:512])
            sin_s = singles.tile([128, T], BF16, tag="sin_s")
            nc.sync.dma_start(out=sin_s[:, 0:512], in_=ap(sin_d)[:, 0:512])
            wvh = load_w(wv_hi_d, "wvh")
            wvl = load_w(wv_lo_d, "wvl")
            nc.sync.dma_start(out=cos_s[:, 512:T], in_=ap(cos_d)[:, 512:T])
            nc.sync.dma_start(out=sin_s[:, 512:T], in_=ap(sin_d)[:, 512:T])
            sk_t = singles.tile([128, TT], F32, tag="sk")
            nc.sync.dma_start(out=sk_t, in_=ap(skt_d))
            for j in range(1, NS):
                load_x_strip(j, True)
                load_x_strip(j, False)
            del wkhv
            mskb = singles.tile([128, 128], BF16, tag="mskb")
            nc.sync.dma_start(out=mskb, in_=ap(mskb_d))
            onc = singles.tile([1, 128], F32R, tag="onc")
            nc.sync.dma_start(out=onc, in_=ap(onc_d))
            on128 = singles.tile([128, 1], BF16, tag="on128")
            nc.sync.dma_start(out=on128, in_=ap(on128_d))
            on2 = singles.tile([128, 2, 64], F8, tag="on2")
            nc.sync.dma_start(out=on2, in_=ap(on2_d).rearrange("p (a b) -> p a b", a=2))
            woh = singles.tile([128, H_LOC, T], F8, tag="woh")
            nc.sync.dma_start(out=woh, in_=ap(wo_hi_d).rearrange("p (h t) -> p h t", h=H_LOC))
            wol = singles.tile([128, H_LOC, T], F8, tag="wol")
            nc.sync.dma_start(out=wol, in_=ap(wo_lo_d).rearrange("p (h t) -> p h t", h=H_LOC))

            # ---------------- projections ------------------------------------------
            q_sb = singles.tile([128, H_LOC, T], BF16, tag="q_sb")
            k_sb = singles.tile([128, H_LOC, T], BF16, tag="k_sb")
            v8 = singles.tile([128, TT, NL], F8, tag="v8")
            v0 = singles.tile([128, 4, NL], BF16, tag="v0")
            outT = singles.tile([128, H_LOC, T], BF16, tag="outT")
            ohi = singles.tile([128, H_LOC, T], F8, tag="ohi")
            olo = singles.tile([128, H_LOC, T], F8, tag="olo")

            def emit_proj_psum(whi, wlo, h, j):
                """Single-group DoubleRow projection at x32: hi*hi + hi*lo +
                lo*hi (residuals stored at the same scale as hi)."""
                hs = slice(h * 128, (h + 1) * 128)
                xh, xl = xhi_t[j], xlo_t[j]
                ps = pA.tile([128, 512], F32, tag="pa")
                for kp in range(KP):
                    ks = slice(2 * kp, 2 * kp + 2)
                    nc.tensor.matmul(
                        ps, lhsT=whi[:, ks, hs], rhs=xh[:, ks, :],
                        start=(kp == 0), stop=False, perf_mode=DR,
                    )
                for kp in range(KP):
                    ks = slice(2 * kp, 2 * kp + 2)
                    nc.tensor.matmul(
                        ps, lhsT=wlo[:, ks, hs], rhs=xh[:, ks, :],
                        start=False, stop=False, perf_mode=DR,
                    )
                for kp in range(KP):
                    ks = slice(2 * kp, 2 * kp + 2)
                    nc.tensor.matmul(
                        ps, lhsT=whi[:, ks, hs], rhs=xl[:, ks, :],
                        start=False, stop=(kp == KP - 1), perf_mode=DR,
                    )
                return ps

            def emit_qk_strip(h, j):
                mark(f"qk h{h} j{j}")
                js = slice(j * 512, (j + 1) * 512)
                for dst, whi, wlo in ((k_sb, wkh, wkl), (q_sb, wqh, wql)):
                    ps = emit_proj_psum(whi, wlo, h, j)
                    # RoPE: rotate-half via a partition-swapping DMA (engines
                    # cannot cross partitions); tables carry s/32 and the sign
                    m1 = rope.tile([128, 512], BF16, tag="m1")
                    nc.vector.tensor_mul(m1, ps, cos_s[:, js])
                    m2 = rope.tile([128, 512], BF16, tag="m2")
                    nc.vector.tensor_mul(m2[0:64, :], ps[64:128, :], sin_s[0:64, js])
                    nc.vector.tensor_mul(m2[64:128, :], ps[0:64, :], sin_s[64:128, js])
                    nc.vector.tensor_add(dst[:, h, js], m1, m2)

            def emit_v_tile(tt):
                # v in [tk, dh] layout directly: lhsT = x (tk columns as the
                # stationary free dim), rhs = wv; no transpose needed.
                mark(f"v tt{tt}")
                j, lt = tt // 4, tt % 4
                ls = slice(lt * 128, (lt + 1) * 128)
                xh, xl = xhi_t[j], xlo_t[j]
                ps = pA.tile([128, NL], F32, tag="pa", name="vps")
                for kp in range(KP):
                    ks = slice(2 * kp, 2 * kp + 2)
                    nc.tensor.matmul(
                        ps, lhsT=xh[:, ks, ls], rhs=wvh[:, ks, :],
                        start=(kp == 0), stop=False, perf_mode=DR,
                    )
                for kp in range(KP):
                    ks = slice(2 * kp, 2 * kp + 2)
                    nc.tensor.matmul(
                        ps, lhsT=xl[:, ks, ls], rhs=wvh[:, ks, :],
                        start=False, stop=False, perf_mode=DR,
                    )
                for kp in range(KP):
                    ks = slice(2 * kp, 2 * kp + 2)
                    nc.tensor.matmul(
                        ps, lhsT=xh[:, ks, ls], rhs=wvl[:, ks, :],
                        start=False, stop=(kp == KP - 1), perf_mode=DR,
                    )
                nc.scalar.activation(v8[:, tt, :], ps, AF.Copy,
                                     scale=sk_t[:, tt : tt + 1])
                if tt < 4:
                    nc.vector.tensor_scalar_mul(v0[:, tt, :], ps, sk_t[:, tt : tt + 1])

            # ---------------- attention --------------------------------------------
            # A phase: all scores + exps of a strip (PE streams scores while
            # Pool/ACT chase with mask + exp). B phase: PV + rowsum + epilogue
            # (runs with a full strip of slack behind the exps).
            e_units = {}
            fin = {}

            def emit_att_A(h, j, fillers=()):
                mark(f"attA h{h} j{j}")
                fillers = list(fillers)
                nunit = 0

                def tick():
                    nonlocal nunit
                    nunit += 1
                    if nunit % 2 == 1 and fillers:
                        fillers.pop(0)()

                js = slice(j * 512, (j + 1) * 512)
                units = []
                if j == 0:
                    for i in range(4):
                        c0 = 128 * i
                        cs = slice(c0, 512)
                        st = psc.tile([128, 512], F32, tag="sc")
                        nc.tensor.matmul(
                            st[:, cs], lhsT=k_sb[:, h, i * 128 : (i + 1) * 128],
                            rhs=q_sb[:, h, cs], start=True, stop=True,
                        )
                        e0 = e0pool.tile([128, 512], BF16, tag="e0")
                        nc.scalar.activation(e0[:, cs], st[:, cs], AF.Exp,
                                             scale=INV_SQRT_DH)
                        nc.gpsimd.tensor_mul(
                            e0[:, c0 : c0 + 128], e0[:, c0 : c0 + 128], mskb
                        )
                        units.append((e0, c0))
                        tick()
                else:
                    npair = 2 * (j + 1)
                    for p_ in range(npair):
                        i0 = 2 * p_
                        r0 = i0 - 4 * j
                        c0p = 128 * r0 if r0 >= 0 else 0
                        e8 = epool.tile([128, 2, 512], F8, tag="e8")
                        for slot in range(2):
                            i = i0 + slot
                            r = i - 4 * j
                            c0 = 128 * r if r > 0 else 0
                            cs = slice(c0, 512)
                            qs = slice(j * 512 + c0, (j + 1) * 512)
                            st = psc.tile([128, 512], F32, tag="sc")
                            nc.tensor.matmul(
                                st[:, cs], lhsT=k_sb[:, h, i * 128 : (i + 1) * 128],
                                rhs=q_sb[:, h, qs], start=True, stop=True,
                            )
                            nc.scalar.activation(e8[:, slot, cs], st[:, cs], AF.Exp,
                                                 scale=INV_SQRT_DH, bias=eoffb)
                            if r >= 0:
                                nc.gpsimd.tensor_mul(
                                    e8[:, slot, c0 : c0 + 128],
                                    e8[:, slot, c0 : c0 + 128], mskb
                                )
                            if slot == 1 and c0 > c0p:
                                nc.gpsimd.memset(e8[:, 1, c0p:c0], 0.0)
                        units.append((e8, c0p))
                        tick()
                e_units[(h, j)] = units
                for f in fillers:
                    f()

            def emit_att_B(h, j):
                mark(f"attB h{h} j{j}")
                hs = slice(h * 128, (h + 1) * 128)
                js = slice(j * 512, (j + 1) * 512)
                units = e_units.pop((h, j))
                po = ppv.tile([128, 512], F32, tag="pv")
                su = psu.tile([64, 512], F32, tag="su")
                n = len(units)
                if j == 0:
                    for i, (e0, c0) in enumerate(units):
                        cs = slice(c0, 512)
                        nc.tensor.matmul(
                            po[:, cs], lhsT=v0[:, i, hs], rhs=e0[:, cs],
                            start=(i == 0), stop=(i == n - 1),
                        )
                        nc.tensor.matmul(
                            su[0:1, cs], lhsT=on128, rhs=e0[:, cs],
                            start=(i == 0), stop=(i == n - 1),
                        )
                else:
                    for p_, (e8, c0p) in enumerate(units):
                        csp = slice(c0p, 512)
                        i0 = 2 * p_
                        nc.tensor.matmul(
                            po[:, csp], lhsT=v8[:, i0 : i0 + 2, hs],
                            rhs=e8[:, :, csp],
                            start=(p_ == 0), stop=(p_ == n - 1), perf_mode=DR,
                        )
                        nc.tensor.matmul(
                            su[:, csp], lhsT=on2, rhs=e8[:, :, csp],
                            start=(p_ == 0), stop=(p_ == n - 1), perf_mode=DR,
                        )  # 64 identical rowsum rows; row 0 is used
                pos = rbsp.tile([128, 512], BF16, tag="pos")
                nc.vector.tensor_copy(pos, po)
                rec = small.tile([1, 512], F32R, tag="rec")
                with nc.allow_low_precision(reason="1/su feeds an f32r bcast"):
                    nc.vector.reciprocal(rec, su[0:1, :])
                fin[(h, j)] = (pos, rec)

            def emit_att_B2(h, j):
                mark(f"attB2 h{h} j{j}")
                js = slice(j * 512, (j + 1) * 512)
                pos, rec = fin.pop((h, j))
                rb = ppv.tile([128, 512], F32, tag="pv", name="rb")
                nc.tensor.matmul(rb, lhsT=onc, rhs=rec, start=True, stop=True)
                nc.vector.tensor_mul(outT[:, h, js], rb, pos)
                nc.gpsimd.tensor_copy(ohi[:, h, js], outT[:, h, js])
                nc.gpsimd.tensor_sub(olo[:, h, js], outT[:, h, js], ohi[:, h, js])

            def emit_att_AB(h, j, fillers=(), lag=5):
                """Last-strip variant: B pairs ride `lag` behind the A units
                so PV/rowsum overlap the score stream instead of trailing it."""
                mark(f"attAB h{h} j{j}")
                fillers = list(fillers)
                hs = slice(h * 128, (h + 1) * 128)
                js = slice(j * 512, (j + 1) * 512)
                npair = 2 * (j + 1)
                po = ppv.tile([128, 512], F32, tag="pv")
                su = psu.tile([64, 512], F32, tag="su")
                units = []

                def emit_B_pair(p_):
                    e8, c0p = units[p_]
                    csp = slice(c0p, 512)
                    i0 = 2 * p_
                    nc.tensor.matmul(
                        po[:, csp], lhsT=v8[:, i0 : i0 + 2, hs],
                        rhs=e8[:, :, csp],
                        start=(p_ == 0), stop=(p_ == npair - 1), perf_mode=DR,
                    )
                    nc.tensor.matmul(
                        su[:, csp], lhsT=on2, rhs=e8[:, :, csp],
                        start=(p_ == 0), stop=(p_ == npair - 1), perf_mode=DR,
                    )

                for p_ in range(npair):
                    i0 = 2 * p_
                    r0 = i0 - 4 * j
                    c0p = 128 * r0 if r0 >= 0 else 0
                    e8 = epool.tile([128, 2, 512], F8, tag="e8")
                    for slot in range(2):
                        i = i0 + slot
                        r = i - 4 * j
                        c0 = 128 * r if r > 0 else 0
                        cs = slice(c0, 512)
                        qs = slice(j * 512 + c0, (j + 1) * 512)
                        st = psc.tile([128, 512], F32, tag="sc")
                        nc.tensor.matmul(
                            st[:, cs], lhsT=k_sb[:, h, i * 128 : (i + 1) * 128],
                            rhs=q_sb[:, h, qs], start=True, stop=True,
                        )
                        nc.scalar.activation(e8[:, slot, cs], st[:, cs], AF.Exp,
                                             scale=INV_SQRT_DH, bias=eoffb)
                        if r >= 0:
                            nc.gpsimd.tensor_mul(
                                e8[:, slot, c0 : c0 + 128],
                                e8[:, slot, c0 : c0 + 128], mskb
                            )
                        if slot == 1 and c0 > c0p:
                            nc.gpsimd.memset(e8[:, 1, c0p:c0], 0.0)
                    units.append((e8, c0p))
                    if p_ % 2 == 1 and fillers:
                        fillers.pop(0)()
                    if p_ >= lag:
                        emit_B_pair(p_ - lag)
                for p_ in range(max(0, npair - lag), npair):
                    emit_B_pair(p_)
                for f in fillers:
                    f()
                pos = rbsp.tile([128, 512], BF16, tag="pos")
                nc.vector.tensor_copy(pos, po)
                rec = small.tile([1, 512], F32R, tag="rec")
                with nc.allow_low_precision(reason="1/su feeds an f32r bcast"):
                    nc.vector.reciprocal(rec, su[0:1, :])
                fin[(h, j)] = (pos, rec)

            def emit_wo_tile(tt, tail=False):
                # fp8 3-chain in one x512 PSUM group: (ohi+olo)*wohi +
                # (outT/16 as fp8)*wolo; host divides by 512
                mark(f"wo tt{tt}")
                ts = slice(tt * 128, (tt + 1) * 128)
                stg = stage.tile([128, T], BF16, tag="stg")
                for n in range(NS):
                    ns = slice(n * 512, (n + 1) * 512)
                    if tail and n % 2 == 1:
                        pa = psc.tile([128, 512], F32, tag="sc", name="wot")
                    else:
                        pa = pA.tile([128, 512], F32, tag="pa")
                    nc.tensor.matmul(
                        pa, lhsT=ohi[:, :, ts], rhs=woh[:, :, ns],
                        start=True, stop=False, perf_mode=DR,
                    )
                    nc.tensor.matmul(
                        pa, lhsT=olo[:, :, ts], rhs=woh[:, :, ns],
                        start=False, stop=False, perf_mode=DR,
                    )
                    nc.tensor.matmul(
                        pa, lhsT=ohi[:, :, ts], rhs=wol[:, :, ns],
                        start=False, stop=True, perf_mode=DR,
                    )
                    if n % 2 == 0:
                        nc.vector.tensor_copy(stg[:, ns], pa)
                    else:
                        nc.scalar.copy(stg[:, ns], pa)
                    if n == 1:
                        nc.sync.dma_start(out=out_ap[ts, 0:1024], in_=stg[:, 0:1024])
                nc.sync.dma_start(out=out_ap[ts, 1024:2048], in_=stg[:, 1024:2048])

            # emission order mirrors data readiness: attention strip j only
            # needs x strips <= j, so each strip's qk/v/attention pipeline
            # rides directly behind its x DMA; wo tiles and deferred
            # epilogues (B2) fill the ACT-paced score streams.
            wo_f = lambda tt: (lambda: emit_wo_tile(tt))
            b2_f = lambda h, j: (lambda: emit_att_B2(h, j))

            emit_qk_strip(0, 0)
            emit_qk_strip(1, 0)
            for tt in range(0, 4):
                emit_v_tile(tt)
            emit_att_A(0, 0)
            emit_att_B(0, 0)
            emit_att_A(1, 0, [b2_f(0, 0)])
            emit_att_B(1, 0)
            emit_qk_strip(0, 1)
            emit_qk_strip(1, 1)
            for tt in range(4, 8):
                emit_v_tile(tt)
            emit_att_A(0, 1, [b2_f(1, 0)])
            emit_att_B(0, 1)
            emit_att_A(1, 1, [b2_f(0, 1)])
            emit_att_B(1, 1)
            emit_qk_strip(0, 2)
            emit_qk_strip(1, 2)
            for tt in range(8, 12):
                emit_v_tile(tt)
            emit_att_A(0, 2, [b2_f(1, 1), wo_f(0), wo_f(1)])
            emit_att_B(0, 2)
            emit_att_A(1, 2, [b2_f(0, 2), wo_f(2), wo_f(3)])
            emit_att_B(1, 2)
            emit_qk_strip(0, 3)
            emit_qk_strip(1, 3)
            for tt in range(12, 16):
                emit_v_tile(tt)
            emit_att_A(0, 3, [b2_f(1, 2), wo_f(4), wo_f(5), wo_f(6)])
            emit_att_B(0, 3)
            emit_att_AB(1, 3, [b2_f(0, 3), wo_f(7), wo_f(8), wo_f(9), wo_f(10),
                               wo_f(11)])
            emit_att_B2(1, 3)
            for tt in (12, 13, 14, 15):
                emit_wo_tile(tt, tail=True)

        for _rep in range(repeats):
            emit_body()

    # Force Exp and Ln onto the single combined table set (one
    # ACT_TABLE_LOAD for the whole kernel).
    from concourse.hw_specs import get_activation_tables
    tabs = get_activation_tables(nc.m.arch)
    for nm_, fs_ in tabs.items():
        if nm_ != "natural_log_exp_and_others":
            fs_.discard(AF.Exp)
            fs_.discard(AF.Ln)
    nc.compile()
    _CACHED[repeats] = nc
    return nc


def _host_prep(x, w_ln, wq, wk, wv, wo, cos, sin):
    bf = ml_dtypes.bfloat16
    f8 = ml_dtypes.float8_e4m3
    x = np.asarray(x, np.float32)
    w_ln = np.asarray(w_ln, np.float32)

    # per-token RMSNorm scale, folded into the RoPE tables and v scale
    s = 1.0 / np.sqrt((x * x).mean(axis=1) + EPS)          # [T] f32

    cosT = np.ascontiguousarray(np.asarray(cos, np.float32).T)   # [D_H, T]
    sinT = np.ascontiguousarray(np.asarray(sin, np.float32).T)
    sinT[0:64] *= -1.0          # rotate_half sign folded into the table
    cos_s = (cosT * (s / WSC)[None, :]).astype(bf)
    sin_s = (sinT * (s / WSC)[None, :]).astype(bf)
    # v8 carries x16 (for the fp8 wo split); proj psum carries x32
    skt = np.ascontiguousarray((s * 16.0 / WSC).reshape(TT, 128).T).astype(np.float32)

    xT = np.ascontiguousarray(x.T).astype(bf).astype(np.float32)
    x_hi = xT.astype(f8)
    x_lo = (xT - x_hi.astype(np.float32)).astype(f8)     # same scale as hi

    def pack_kd(a, ncols):
        # [D, M] -> [128, KD*M]: tile[p, kd*M + m] = a[kd*128 + p, m]
        return np.ascontiguousarray(
            a.reshape(KD, 128, ncols).transpose(1, 0, 2).reshape(128, KD * ncols))

    xhi_p = {f"xhi{j}": pack_kd(x_hi[:, j * 512:(j + 1) * 512], 512) for j in range(NS)}
    xlo_p = {f"xlo{j}": pack_kd(x_lo[:, j * 512:(j + 1) * 512], 512) for j in range(NS)}

    # causal boundary: 1 where tq >= tk within the tile, else 0
    f = np.arange(128)[None, :]
    p = np.arange(128)[:, None]
    maskb = (f >= p).astype(bf)

    ones_col = np.ones((1, 128), np.float32)
    ones128 = np.ones((128, 1), bf)
    ones2f8 = np.ones((128, 128), f8)

    wq_s = np.asarray(wq, np.float32) * w_ln[None, :]
    wk_s = np.asarray(wk, np.float32) * w_ln[None, :]
    wv_s = np.asarray(wv, np.float32) * w_ln[None, :]
    wo32 = np.asarray(wo, np.float32)

    def wsplit(w_sl):
        # [D, NL] slice, bf16-rounded like the reference weights path;
        # hi and the residual lo are stored at the same x32 scale
        wT = np.ascontiguousarray(w_sl.T).astype(bf).astype(np.float32) * WSC
        hi = wT.astype(f8)
        lo = (wT - hi.astype(np.float32)).astype(f8)
        return pack_kd(hi, NL), pack_kd(lo, NL)

    in_maps = []
    for c in range(N_CORES):
        sl = slice(c * NL, (c + 1) * NL)
        wqh, wql = wsplit(wq_s[sl])
        wkh, wkl = wsplit(wk_s[sl])
        wvh, wvl = wsplit(wv_s[sl])
        woT = np.ascontiguousarray(wo32[:, sl].T).astype(bf).astype(np.float32) * WSC
        woT_hi = woT.astype(f8)
        woT_lo = (woT - woT_hi.astype(np.float32)).astype(f8)

        def pack_h(a):
            return np.ascontiguousarray(
                a.reshape(H_LOC, 128, T).transpose(1, 0, 2).reshape(128, H_LOC * T))

        in_maps.append({
            **xhi_p, **xlo_p,
            "wqhi": wqh, "wqlo": wql,
            "wkhi": wkh, "wklo": wkl,
            "wvhi": wvh, "wvlo": wvl,
            "wohi": pack_h(woT_hi), "wolo": pack_h(woT_lo),
            "cosT": cos_s,
            "sinT": sin_s,
            "skt": skt,
            "maskb": maskb,
            "ones_col": ones_col,
            "ones128": ones128,
            "ones2f8": ones2f8,
        })
    return in_maps


def kernel(x, w_ln, wq, wk, wv, wo, cos, sin):
    nc = _build_program()
    in_maps = _host_prep(x, w_ln, wq, wk, wv, wo, cos, sin)
    t0 = time.time()
    res = run_bass_kernel_spmd(nc, in_maps, core_ids=list(range(N_CORES)))
    t1 = time.time()
    print(f"run_bass_kernel_spmd wall: {(t1 - t0) * 1e3:.1f} ms", file=sys.stderr)
    acc = np.zeros((T, D), np.float32)
    for r in res.results:
        acc += np.asarray(r["out"], np.float32)
    return np.asarray(x, np.float32) + acc * (1.0 / OSC)


# revision 62
# speedup vs baseline: 769.4526x; 1.0007x over previous
"""Trainium2 Bass kernel for nn_Attention (T=2048, D=2048, H=16, Dh=128).

Strategy: tensor-parallel over heads, 2 heads per core on 8 cores.
fp8 (e4m3) DoubleRow matmuls everywhere precision allows, with same-scale
residual compensation so the result keeps bf16-level accuracy:

  - host prep: w_ln and the per-token RMSNorm scale s fold into the RoPE
    tables / v scale; every operand is split into fp8 hi + residual lo at
    the SAME scale (w at x32, x at x1) and prepacked partition-major so
    all DMAs are 128 long contiguous runs
  - projections q/k/v: three DoubleRow chains (hi*hi + lo*hi + hi*lo)
    accumulate in ONE x32 PSUM group = ~bf16 accuracy at 75% of the bf16
    matmul cost; RoPE reads the PSUM directly (rotate-half crosses
    partitions, legal only for PSUM sources); v is computed directly in
    [tk, dh] layout (lhsT = x) so no transpose is needed
  - attention: transposed-score layout S^T[tk,tq], scores bf16, causal
    mask as a post-exp 0/1 multiply on Pool; strip j=0 keeps bf16 probs;
    strips 1-3 exp to fp8 with a -4.5 offset (cancels in the deferred
    softmax normalization) and run PV + rowsum as fp8 DoubleRow over
    tk-tile pairs (the rowsum ones-vector is 64 wide - dual-fp8
    Ldweights rejects narrow weight tiles)
  - wo: fp8 3-chain DoubleRow with heads as the pair dim; outT splits to
    hi + lo on device; partials staged bf16 at x512, host sums in f32
  - schedule: per-strip software pipeline (scores+exps phase, then PV
    phase with a full strip of slack), wo tiles and deferred epilogues
    ride as fillers inside the ACT-paced score streams, PE warmup
    matmuls bridge the initial DMA wait to keep the p-state ramp hot
"""

import math
import os
import sys
import time

for _p in ("/opt/trn_rl_repo", "/root/.axon_site/_ro/trn_rl_repo"):
    if os.path.isdir(_p) and _p not in sys.path:
        sys.path.insert(0, _p)

import numpy as np
import ml_dtypes

import concourse.bass as bass
import concourse.tile as tile
from concourse import bacc, mybir
from concourse.bass_utils import run_bass_kernel_spmd

BF16 = mybir.dt.bfloat16
F8 = mybir.dt.float8e4
F32R = mybir.dt.float32r
F32 = mybir.dt.float32
AF = mybir.ActivationFunctionType
ALU = mybir.AluOpType
DR = mybir.MatmulPerfMode.DoubleRow

T = 2048
D = 2048
N_H = 16
D_H = 128
N_CORES = 8
H_LOC = N_H // N_CORES          # heads per core = 2
NL = H_LOC * D_H                # local head width = 256
KD = D // 128                   # contraction tiles = 16
KP = KD // 2                    # DoubleRow contraction pairs = 8
TT = T // 128                   # t tiles = 16
NS = T // 512                   # 512-wide strips = 4
EPS = 1e-5
INV_SQRT_DH = 1.0 / math.sqrt(D_H)
EOFF = 4.5                      # fp8 exp offset (cancels in normalization)
WSC = 32.0                      # fp8 weight scale (hi and lo residual alike)
OSC = 512.0                     # output scale (16 from v, 32 from wo weights)

_CACHED = {}
_MARKS = []          # (label, next-instruction-name) pairs when _MARK_ON
_MARK_ON = [False]   # analysis hook; stays off in normal runs


def _build_program(repeats=1):
    if repeats in _CACHED:
        return _CACHED[repeats]

    nc = bacc.Bacc("TRN2", target_bir_lowering=False, debug=False, num_devices=N_CORES)

    # all bulk inputs host-prepacked to partition-major [128, ...] so every
    # DMA is 128 long contiguous runs
    xhi_ds = [nc.dram_tensor(f"xhi{j}", [128, KD * 512], F8, kind="ExternalInput")
              for j in range(NS)]
    xlo_ds = [nc.dram_tensor(f"xlo{j}", [128, KD * 512], F8, kind="ExternalInput")
              for j in range(NS)]
    wq_hi_d = nc.dram_tensor("wqhi", [128, KD * NL], F8, kind="ExternalInput")
    wq_lo_d = nc.dram_tensor("wqlo", [128, KD * NL], F8, kind="ExternalInput")
    wk_hi_d = nc.dram_tensor("wkhi", [128, KD * NL], F8, kind="ExternalInput")
    wk_lo_d = nc.dram_tensor("wklo", [128, KD * NL], F8, kind="ExternalInput")
    wv_hi_d = nc.dram_tensor("wvhi", [128, KD * NL], F8, kind="ExternalInput")
    wv_lo_d = nc.dram_tensor("wvlo", [128, KD * NL], F8, kind="ExternalInput")
    wo_hi_d = nc.dram_tensor("wohi", [128, H_LOC * T], F8, kind="ExternalInput")
    wo_lo_d = nc.dram_tensor("wolo", [128, H_LOC * T], F8, kind="ExternalInput")
    cos_d = nc.dram_tensor("cosT", [D_H, T], BF16, kind="ExternalInput")
    sin_d = nc.dram_tensor("sinT", [D_H, T], BF16, kind="ExternalInput")
    skt_d = nc.dram_tensor("skt", [128, TT], F32, kind="ExternalInput")
    mskb_d = nc.dram_tensor("maskb", [128, 128], BF16, kind="ExternalInput")
    onc_d = nc.dram_tensor("ones_col", [1, 128], F32R, kind="ExternalInput")
    on128_d = nc.dram_tensor("ones128", [128, 1], BF16, kind="ExternalInput")
    on2_d = nc.dram_tensor("ones2f8", [128, 128], F8, kind="ExternalInput")
    out_d = nc.dram_tensor("out", [T, D], BF16, kind="ExternalOutput")

    ap = lambda h: h.ap()
    out_ap = ap(out_d)

    from contextlib import ExitStack

    with tile.TileContext(nc) as tc, ExitStack() as ctx:
        P = ctx.enter_context  # noqa

        singles = P(tc.tile_pool(name="singles", bufs=1))
        rope = P(tc.tile_pool(name="rope", bufs=4))        # [128,512] bf16
        epool = P(tc.tile_pool(name="epool", bufs=10))     # [128,2,512] f8 pairs
        e0pool = P(tc.tile_pool(name="e0pool", bufs=8))    # [128,512] bf16 strip0
        rbsp = P(tc.tile_pool(name="rbsp", bufs=2))        # [128,512] bf16 pv evac
        small = P(tc.tile_pool(name="small", bufs=2))      # [1,512] f32
        stage = P(tc.tile_pool(name="stage", bufs=4))      # [128,T] bf16 out staging
        pA = P(tc.tile_pool(name="pA", bufs=3, space="PSUM"))   # proj + wo
        psc = P(tc.tile_pool(name="psc", bufs=3, space="PSUM")) # scores + tail wo
        ppv = P(tc.tile_pool(name="ppv", bufs=1, space="PSUM")) # po + rb
        psu = P(tc.tile_pool(name="psu", bufs=1, space="PSUM")) # su

        def mark(label):
            if _MARK_ON[0]:
                _MARKS.append((label, nc.get_next_instruction_name()))

        def emit_body():
            mark("loads")
            # ---------------- phase 0: loads in consumption order ------------------
            # PE warmup: ~5us of tiny matmuls during the initial DMA wait so the
            # p-state ramp completes before real work arrives
            warm = singles.tile([128, 256], BF16, tag="warm")
            nc.gpsimd.memset(warm, 1.0)
            eoffb = singles.tile([128, 1], F32, tag="eoffb")
            nc.vector.memset(eoffb, -EOFF)
            for _w in range(16):
                pw = psc.tile([128, 256], F32, tag="sc", name=f"warm{_w}")
                nc.tensor.matmul(pw, lhsT=warm[:, 0:128], rhs=warm,
                                 start=True, stop=True)

            xhi_t = [
                singles.tile([128, KD, 512], F8, tag=f"xhi{j}", name=f"xhi{j}")
                for j in range(NS)
            ]
            xlo_t = [
                singles.tile([128, KD, 512], F8, tag=f"xlo{j}", name=f"xlo{j}")
                for j in range(NS)
            ]

            def load_x_strip(j, hi, half=None):
                t_, d_ = (xhi_t[j], xhi_ds[j]) if hi else (xlo_t[j], xlo_ds[j])
                dv = ap(d_).rearrange("p (a m) -> p a m", a=KD)
                if half is None:
                    nc.sync.dma_start(out=t_, in_=dv)
                elif half == 0:
                    nc.sync.dma_start(out=t_[:, 0:8, :], in_=dv[:, 0:8, :])
                else:
                    nc.sync.dma_start(out=t_[:, 8:16, :], in_=dv[:, 8:16, :])

            def load_w(dram, tag, split=False):
                t_ = singles.tile([128, KD, NL], F8, tag=tag)
                dv = ap(dram).rearrange("p (a m) -> p a m", a=KD)
                if split:
                    nc.sync.dma_start(out=t_[:, 0:8, :], in_=dv[:, 0:8, :])
                    nc.sync.dma_start(out=t_[:, 8:16, :], in_=dv[:, 8:16, :])
                else:
                    nc.sync.dma_start(out=t_, in_=dv)
                return t_

            # interleave x0/w halves so the first chains start ~3us in
            wkh = singles.tile([128, KD, NL], F8, tag="wkh")
            wkhv = ap(wk_hi_d).rearrange("p (a m) -> p a m", a=KD)
            nc.sync.dma_start(out=wkh[:, 0:8, :], in_=wkhv[:, 0:8, :])
            load_x_strip(0, True, 0)
            nc.sync.dma_start(out=wkh[:, 8:16, :], in_=wkhv[:, 8:16, :])
            load_x_strip(0, True, 1)
            wkl = load_w(wk_lo_d, "wkl", split=True)
            load_x_strip(0, False, 0)
            load_x_strip(0, False, 1)
            wqh = load_w(wq_hi_d, "wqh", split=True)
            wql = load_w(wq_lo_d, "wql", split=True)
            # strip-0 table columns first: RoPE j0 unblocks before the wv bulk
            cos_s = singles.tile([128, T], BF16, tag="cos_s")
            nc.sync.dma_start(out=cos_s[:, 0:512], in_=ap(cos_d)[:, 0:512])
            sin_s = singles.tile([128, T], BF16, tag="sin_s")
            nc.sync.dma_start(out=sin_s[:, 0:512], in_=ap(sin_d)[:, 0:512])
            wvh = load_w(wv_hi_d, "wvh")
            wvl = load_w(wv_lo_d, "wvl")
            nc.sync.dma_start(out=cos_s[:, 512:T], in_=ap(cos_d)[:, 512:T])
            nc.sync.dma_start(out=sin_s[:, 512:T], in_=ap(sin_d)[:, 512:T])
            sk_t = singles.tile([128, TT], F32, tag="sk")
            nc.sync.dma_start(out=sk_t, in_=ap(skt_d))
            for j in range(1, NS):
                load_x_strip(j, True)
                load_x_strip(j, False)
            del wkhv
            mskb = singles.tile([128, 128], BF16, tag="mskb")
            nc.sync.dma_start(out=mskb, in_=ap(mskb_d))
            onc = singles.tile([1, 128], F32R, tag="onc")
            nc.sync.dma_start(out=onc, in_=ap(onc_d))
            on128 = singles.tile([128, 1], BF16, tag="on128")
            nc.sync.dma_start(out=on128, in_=ap(on128_d))
            on2 = singles.tile([128, 2, 64], F8, tag="on2")
            nc.sync.dma_start(out=on2, in_=ap(on2_d).rearrange("p (a b) -> p a b", a=2))
            woh = singles.tile([128, H_LOC, T], F8, tag="woh")
            nc.sync.dma_start(out=woh, in_=ap(wo_hi_d).rearrange("p (h t) -> p h t", h=H_LOC))
            wol = singles.tile([128, H_LOC, T], F8, tag="wol")
            nc.sync.dma_start(out=wol, in_=ap(wo_lo_d).rearrange("p (h t) -> p h t", h=H_LOC))

            # ---------------- projections ------------------------------------------
            q_sb = singles.tile([128, H_LOC, T], BF16, tag="q_sb")
            k_sb = singles.tile([128, H_LOC, T], BF16, tag="k_sb")
            v8 = singles.tile([128, TT, NL], F8, tag="v8")
            v0 = singles.tile([128, 4, NL], BF16, tag="v0")
            outT = singles.tile([128, H_LOC, T], BF16, tag="outT")
            ohi = singles.tile([128, H_LOC, T], F8, tag="ohi")
            olo = singles.tile([128, H_LOC, T], F8, tag="olo")

            def emit_proj_psum(whi, wlo, h, j):
                """Single-group DoubleRow projection at x32: hi*hi + hi*lo +
                lo*hi (residuals stored at the same scale as hi)."""
                hs = slice(h * 128, (h + 1) * 128)
                xh, xl = xhi_t[j], xlo_t[j]
                ps = pA.tile([128, 512], F32, tag="pa")
                for kp in range(KP):
                    ks = slice(2 * kp, 2 * kp + 2)
                    nc.tensor.matmul(
                        ps, lhsT=whi[:, ks, hs], rhs=xh[:, ks, :],
                        start=(kp == 0), stop=False, perf_mode=DR,
                    )
                for kp in range(KP):
                    ks = slice(2 * kp, 2 * kp + 2)
                    nc.tensor.matmul(
                        ps, lhsT=wlo[:, ks, hs], rhs=xh[:, ks, :],
                        start=False, stop=False, perf_mode=DR,
                    )
                for kp in range(KP):
                    ks = slice(2 * kp, 2 * kp + 2)
                    nc.tensor.matmul(
                        ps, lhsT=whi[:, ks, hs], rhs=xl[:, ks, :],
                        start=False, stop=(kp == KP - 1), perf_mode=DR,
                    )
                return ps

            def emit_qk_strip(h, j):
                mark(f"qk h{h} j{j}")
                js = slice(j * 512, (j + 1) * 512)
                for dst, whi, wlo in ((k_sb, wkh, wkl), (q_sb, wqh, wql)):
                    ps = emit_proj_psum(whi, wlo, h, j)
                    # RoPE: rotate-half via a partition-swapping DMA (engines
                    # cannot cross partitions); tables carry s/32 and the sign
                    m1 = rope.tile([128, 512], BF16, tag="m1")
                    nc.vector.tensor_mul(m1, ps, cos_s[:, js])
                    m2 = rope.tile([128, 512], BF16, tag="m2")
                    nc.vector.tensor_mul(m2[0:64, :], ps[64:128, :], sin_s[0:64, js])
                    nc.vector.tensor_mul(m2[64:128, :], ps[0:64, :], sin_s[64:128, js])
                    nc.vector.tensor_add(dst[:, h, js], m1, m2)

            def emit_qk_pair0():
                # strip-0 variant: k chains for both heads first (their
                # weights arrive before wq), then q chains
                mark("qk pair j0")
                js = slice(0, 512)
                for dst, whi, wlo in ((k_sb, wkh, wkl), (q_sb, wqh, wql)):
                    for h in range(H_LOC):
                        ps = emit_proj_psum(whi, wlo, h, 0)
                        m1 = rope.tile([128, 512], BF16, tag="m1")
                        nc.vector.tensor_mul(m1, ps, cos_s[:, js])
                        m2 = rope.tile([128, 512], BF16, tag="m2")
                        nc.vector.tensor_mul(m2[0:64, :], ps[64:128, :],
                                             sin_s[0:64, js])
                        nc.vector.tensor_mul(m2[64:128, :], ps[0:64, :],
                                             sin_s[64:128, js])
                        nc.vector.tensor_add(dst[:, h, js], m1, m2)

            def emit_v_tile(tt):
                # v in [tk, dh] layout directly: lhsT = x (tk columns as the
                # stationary free dim), rhs = wv; no transpose needed.
                mark(f"v tt{tt}")
                j, lt = tt // 4, tt % 4
                ls = slice(lt * 128, (lt + 1) * 128)
                xh, xl = xhi_t[j], xlo_t[j]
                ps = pA.tile([128, NL], F32, tag="pa", name="vps")
                for kp in range(KP):
                    ks = slice(2 * kp, 2 * kp + 2)
                    nc.tensor.matmul(
                        ps, lhsT=xh[:, ks, ls], rhs=wvh[:, ks, :],
                        start=(kp == 0), stop=False, perf_mode=DR,
                    )
                for kp in range(KP):
                    ks = slice(2 * kp, 2 * kp + 2)
                    nc.tensor.matmul(
                        ps, lhsT=xl[:, ks, ls], rhs=wvh[:, ks, :],
                        start=False, stop=False, perf_mode=DR,
                    )
                for kp in range(KP):
                    ks = slice(2 * kp, 2 * kp + 2)
                    nc.tensor.matmul(
                        ps, lhsT=xh[:, ks, ls], rhs=wvl[:, ks, :],
                        start=False, stop=(kp == KP - 1), perf_mode=DR,
                    )
                nc.scalar.activation(v8[:, tt, :], ps, AF.Copy,
                                     scale=sk_t[:, tt : tt + 1])
                if tt < 4:
                    nc.vector.tensor_scalar_mul(v0[:, tt, :], ps, sk_t[:, tt : tt + 1])

            # ---------------- attention --------------------------------------------
            # A phase: all scores + exps of a strip (PE streams scores while
            # Pool/ACT chase with mask + exp). B phase: PV + rowsum + epilogue
            # (runs with a full strip of slack behind the exps).
            e_units = {}
            fin = {}

            def emit_att_A(h, j, fillers=()):
                mark(f"attA h{h} j{j}")
                fillers = list(fillers)
                nunit = 0

                def tick():
                    nonlocal nunit
                    nunit += 1
                    if nunit % 2 == 1 and fillers:
                        fillers.pop(0)()

                js = slice(j * 512, (j + 1) * 512)
                units = []
                if j == 0:
                    for i in range(4):
                        c0 = 128 * i
                        cs = slice(c0, 512)
                        st = psc.tile([128, 512], F32, tag="sc")
                        nc.tensor.matmul(
                            st[:, cs], lhsT=k_sb[:, h, i * 128 : (i + 1) * 128],
                            rhs=q_sb[:, h, cs], start=True, stop=True,
                        )
                        e0 = e0pool.tile([128, 512], BF16, tag="e0")
                        nc.scalar.activation(e0[:, cs], st[:, cs], AF.Exp,
                                             scale=INV_SQRT_DH)
                        nc.gpsimd.tensor_mul(
                            e0[:, c0 : c0 + 128], e0[:, c0 : c0 + 128], mskb
                        )
                        units.append((e0, c0))
                        tick()
                else:
                    npair = 2 * (j + 1)
                    for p_ in range(npair):
                        i0 = 2 * p_
                        r0 = i0 - 4 * j
                        c0p = 128 * r0 if r0 >= 0 else 0
                        e8 = epool.tile([128, 2, 512], F8, tag="e8")
                        for slot in range(2):
                            i = i0 + slot
                            r = i - 4 * j
                            c0 = 128 * r if r > 0 else 0
                            cs = slice(c0, 512)
                            qs = slice(j * 512 + c0, (j + 1) * 512)
                            st = psc.tile([128, 512], F32, tag="sc")
                            nc.tensor.matmul(
                                st[:, cs], lhsT=k_sb[:, h, i * 128 : (i + 1) * 128],
                                rhs=q_sb[:, h, qs], start=True, stop=True,
                            )
                            nc.scalar.activation(e8[:, slot, cs], st[:, cs], AF.Exp,
                                                 scale=INV_SQRT_DH, bias=eoffb)
                            if r >= 0:
                                nc.gpsimd.tensor_mul(
                                    e8[:, slot, c0 : c0 + 128],
                                    e8[:, slot, c0 : c0 + 128], mskb
                                )
                            if slot == 1 and c0 > c0p:
                                nc.gpsimd.memset(e8[:, 1, c0p:c0], 0.0)
                        units.append((e8, c0p))
                        tick()
                e_units[(h, j)] = units
                for f in fillers:
                    f()

            def emit_att_B(h, j):
                mark(f"attB h{h} j{j}")
                hs = slice(h * 128, (h + 1) * 128)
                js = slice(j * 512, (j + 1) * 512)
                units = e_units.pop((h, j))
                po = ppv.tile([128, 512], F32, tag="pv")
                su = psu.tile([64, 512], F32, tag="su")
                n = len(units)
                if j == 0:
                    for i, (e0, c0) in enumerate(units):
                        cs = slice(c0, 512)
                        nc.tensor.matmul(
                            po[:, cs], lhsT=v0[:, i, hs], rhs=e0[:, cs],
                            start=(i == 0), stop=(i == n - 1),
                        )
                        nc.tensor.matmul(
                            su[0:1, cs], lhsT=on128, rhs=e0[:, cs],
                            start=(i == 0), stop=(i == n - 1),
                        )
                else:
                    for p_, (e8, c0p) in enumerate(units):
                        csp = slice(c0p, 512)
                        i0 = 2 * p_
                        nc.tensor.matmul(
                            po[:, csp], lhsT=v8[:, i0 : i0 + 2, hs],
                            rhs=e8[:, :, csp],
                            start=(p_ == 0), stop=(p_ == n - 1), perf_mode=DR,
                        )
                        nc.tensor.matmul(
                            su[:, csp], lhsT=on2, rhs=e8[:, :, csp],
                            start=(p_ == 0), stop=(p_ == n - 1), perf_mode=DR,
                        )  # 64 identical rowsum rows; row 0 is used
                pos = rbsp.tile([128, 512], BF16, tag="pos")
                nc.vector.tensor_copy(pos, po)
                rec = small.tile([1, 512], F32R, tag="rec")
                with nc.allow_low_precision(reason="1/su feeds an f32r bcast"):
                    nc.vector.reciprocal(rec, su[0:1, :])
                fin[(h, j)] = (pos, rec)

            def emit_att_B2(h, j):
                mark(f"attB2 h{h} j{j}")
                js = slice(j * 512, (j + 1) * 512)
                pos, rec = fin.pop((h, j))
                rb = ppv.tile([128, 512], F32, tag="pv", name="rb")
                nc.tensor.matmul(rb, lhsT=onc, rhs=rec, start=True, stop=True)
                nc.vector.tensor_mul(outT[:, h, js], rb, pos)
                nc.gpsimd.tensor_copy(ohi[:, h, js], outT[:, h, js])
                nc.gpsimd.tensor_sub(olo[:, h, js], outT[:, h, js], ohi[:, h, js])

            def emit_att_AB(h, j, fillers=(), lag=5):
                """Last-strip variant: B pairs ride `lag` behind the A units
                so PV/rowsum overlap the score stream instead of trailing it."""
                mark(f"attAB h{h} j{j}")
                fillers = list(fillers)
                hs = slice(h * 128, (h + 1) * 128)
                js = slice(j * 512, (j + 1) * 512)
                npair = 2 * (j + 1)
                po = ppv.tile([128, 512], F32, tag="pv")
                su = psu.tile([64, 512], F32, tag="su")
                units = []

                def emit_B_pair(p_):
                    e8, c0p = units[p_]
                    csp = slice(c0p, 512)
                    i0 = 2 * p_
                    nc.tensor.matmul(
                        po[:, csp], lhsT=v8[:, i0 : i0 + 2, hs],
                        rhs=e8[:, :, csp],
                        start=(p_ == 0), stop=(p_ == npair - 1), perf_mode=DR,
                    )
                    nc.tensor.matmul(
                        su[:, csp], lhsT=on2, rhs=e8[:, :, csp],
                        start=(p_ == 0), stop=(p_ == npair - 1), perf_mode=DR,
                    )

                for p_ in range(npair):
                    i0 = 2 * p_
                    r0 = i0 - 4 * j
                    c0p = 128 * r0 if r0 >= 0 else 0
                    e8 = epool.tile([128, 2, 512], F8, tag="e8")
                    for slot in range(2):
                        i = i0 + slot
                        r = i - 4 * j
                        c0 = 128 * r if r > 0 else 0
                        cs = slice(c0, 512)
                        qs = slice(j * 512 + c0, (j + 1) * 512)
                        st = psc.tile([128, 512], F32, tag="sc")
                        nc.tensor.matmul(
                            st[:, cs], lhsT=k_sb[:, h, i * 128 : (i + 1) * 128],
                            rhs=q_sb[:, h, qs], start=True, stop=True,
                        )
                        nc.scalar.activation(e8[:, slot, cs], st[:, cs], AF.Exp,
                                             scale=INV_SQRT_DH, bias=eoffb)
                        if r >= 0:
                            nc.gpsimd.tensor_mul(
                                e8[:, slot, c0 : c0 + 128],
                                e8[:, slot, c0 : c0 + 128], mskb
                            )
                        if slot == 1 and c0 > c0p:
                            nc.gpsimd.memset(e8[:, 1, c0p:c0], 0.0)
                    units.append((e8, c0p))
                    if p_ % 2 == 1 and fillers:
                        fillers.pop(0)()
                    if p_ >= lag:
                        emit_B_pair(p_ - lag)
                for p_ in range(max(0, npair - lag), npair):
                    emit_B_pair(p_)
                for f in fillers:
                    f()
                pos = rbsp.tile([128, 512], BF16, tag="pos")
                nc.vector.tensor_copy(pos, po)
                rec = small.tile([1, 512], F32R, tag="rec")
                with nc.allow_low_precision(reason="1/su feeds an f32r bcast"):
                    nc.vector.reciprocal(rec, su[0:1, :])
                fin[(h, j)] = (pos, rec)

            def emit_wo_tile(tt, tail=False):
                # fp8 3-chain in one x512 PSUM group: (ohi+olo)*wohi +
                # (outT/16 as fp8)*wolo; host divides by 512
                mark(f"wo tt{tt}")
                ts = slice(tt * 128, (tt + 1) * 128)
                stg = stage.tile([128, T], BF16, tag="stg")
                for n in range(NS):
                    ns = slice(n * 512, (n + 1) * 512)
                    if tail and n % 2 == 1:
                        pa = psc.tile([128, 512], F32, tag="sc", name="wot")
                    else:
                        pa = pA.tile([128, 512], F32, tag="pa")
                    nc.tensor.matmul(
                        pa, lhsT=ohi[:, :, ts], rhs=woh[:, :, ns],
                        start=True, stop=False, perf_mode=DR,
                    )
                    nc.tensor.matmul(
                        pa, lhsT=olo[:, :, ts], rhs=woh[:, :, ns],
                        start=False, stop=False, perf_mode=DR,
                    )
                    nc.tensor.matmul(
                        pa, lhsT=ohi[:, :, ts], rhs=wol[:, :, ns],
                        start=False, stop=True, perf_mode=DR,
                    )
                    if n % 2 == 0:
                        nc.vector.tensor_copy(stg[:, ns], pa)
                    else:
                        nc.scalar.copy(stg[:, ns], pa)
                    if n == 1:
                        nc.sync.dma_start(out=out_ap[ts, 0:1024], in_=stg[:, 0:1024])
                nc.sync.dma_start(out=out_ap[ts, 1024:2048], in_=stg[:, 1024:2048])

            # emission order mirrors data readiness: attention strip j only
            # needs x strips <= j, so each strip's qk/v/attention pipeline
            # rides directly behind its x DMA; wo tiles and deferred
            # epilogues (B2) fill the ACT-paced score streams.
            wo_f = lambda tt: (lambda: emit_wo_tile(tt))
            b2_f = lambda h, j: (lambda: emit_att_B2(h, j))

            emit_qk_pair0()
            for tt in range(0, 4):
                emit_v_tile(tt)
            emit_att_A(0, 0)
            emit_att_B(0, 0)
            emit_att_A(1, 0, [b2_f(0, 0)])
            emit_att_B(1, 0)
            emit_qk_strip(0, 1)
            emit_qk_strip(1, 1)
            for tt in range(4, 8):
                emit_v_tile(tt)
            emit_att_A(0, 1, [b2_f(1, 0)])
            emit_att_B(0, 1)
            emit_att_A(1, 1, [b2_f(0, 1)])
            emit_att_B(1, 1)
            emit_qk_strip(0, 2)
            emit_qk_strip(1, 2)
            for tt in range(8, 12):
                emit_v_tile(tt)
            emit_att_A(0, 2, [b2_f(1, 1), wo_f(0), wo_f(1)])
            emit_att_B(0, 2)
            emit_att_A(1, 2, [b2_f(0, 2), wo_f(2), wo_f(3)])
            emit_att_B(1, 2)
            emit_qk_strip(0, 3)
            emit_qk_strip(1, 3)
            for tt in range(12, 16):
                emit_v_tile(tt)
            emit_att_A(0, 3, [b2_f(1, 2), wo_f(4), wo_f(5), wo_f(6)])
            emit_att_B(0, 3)
            emit_att_AB(1, 3, [b2_f(0, 3), wo_f(7), wo_f(8), wo_f(9), wo_f(10),
                               wo_f(11)])
            emit_att_B2(1, 3)
            for tt in (12, 13, 14, 15):
                emit_wo_tile(tt, tail=True)

        for _rep in range(repeats):
            emit_body()

    # Force Exp and Ln onto the single combined table set (one
    # ACT_TABLE_LOAD for the whole kernel).
    from concourse.hw_specs import get_activation_tables
    tabs = get_activation_tables(nc.m.arch)
    for nm_, fs_ in tabs.items():
        if nm_ != "natural_log_exp_and_others":
            fs_.discard(AF.Exp)
            fs_.discard(AF.Ln)
    nc.compile()
    _CACHED[repeats] = nc
    return nc


def _host_prep(x, w_ln, wq, wk, wv, wo, cos, sin):
    bf = ml_dtypes.bfloat16
    f8 = ml_dtypes.float8_e4m3
    x = np.asarray(x, np.float32)
    w_ln = np.asarray(w_ln, np.float32)

    # per-token RMSNorm scale, folded into the RoPE tables and v scale
    s = 1.0 / np.sqrt((x * x).mean(axis=1) + EPS)          # [T] f32

    cosT = np.ascontiguousarray(np.asarray(cos, np.float32).T)   # [D_H, T]
    sinT = np.ascontiguousarray(np.asarray(sin, np.float32).T)
    sinT[0:64] *= -1.0          # rotate_half sign folded into the table
    cos_s = (cosT * (s / WSC)[None, :]).astype(bf)
    sin_s = (sinT * (s / WSC)[None, :]).astype(bf)
    # v8 carries x16 (for the fp8 wo split); proj psum carries x32
    skt = np.ascontiguousarray((s * 16.0 / WSC).reshape(TT, 128).T).astype(np.float32)

    xT = np.ascontiguousarray(x.T).astype(bf).astype(np.float32)
    x_hi = xT.astype(f8)
    x_lo = (xT - x_hi.astype(np.float32)).astype(f8)     # same scale as hi

    def pack_kd(a, ncols):
        # [D, M] -> [128, KD*M]: tile[p, kd*M + m] = a[kd*128 + p, m]
        return np.ascontiguousarray(
            a.reshape(KD, 128, ncols).transpose(1, 0, 2).reshape(128, KD * ncols))

    xhi_p = {f"xhi{j}": pack_kd(x_hi[:, j * 512:(j + 1) * 512], 512) for j in range(NS)}
    xlo_p = {f"xlo{j}": pack_kd(x_lo[:, j * 512:(j + 1) * 512], 512) for j in range(NS)}

    # causal boundary: 1 where tq >= tk within the tile, else 0
    f = np.arange(128)[None, :]
    p = np.arange(128)[:, None]
    maskb = (f >= p).astype(bf)

    ones_col = np.ones((1, 128), np.float32)
    ones128 = np.ones((128, 1), bf)
    ones2f8 = np.ones((128, 128), f8)

    wq_s = np.asarray(wq, np.float32) * w_ln[None, :]
    wk_s = np.asarray(wk, np.float32) * w_ln[None, :]
    wv_s = np.asarray(wv, np.float32) * w_ln[None, :]
    wo32 = np.asarray(wo, np.float32)

    def wsplit(w_sl):
        # [D, NL] slice, bf16-rounded like the reference weights path;
        # hi and the residual lo are stored at the same x32 scale
        wT = np.ascontiguousarray(w_sl.T).astype(bf).astype(np.float32) * WSC
        hi = wT.astype(f8)
        lo = (wT - hi.astype(np.float32)).astype(f8)
        return pack_kd(hi, NL), pack_kd(lo, NL)

    in_maps = []
    for c in range(N_CORES):
        sl = slice(c * NL, (c + 1) * NL)
        wqh, wql = wsplit(wq_s[sl])
        wkh, wkl = wsplit(wk_s[sl])
        wvh, wvl = wsplit(wv_s[sl])
        woT = np.ascontiguousarray(wo32[:, sl].T).astype(bf).astype(np.float32) * WSC
        woT_hi = woT.astype(f8)
        woT_lo = (woT - woT_hi.astype(np.float32)).astype(f8)

        def pack_h(a):
            return np.ascontiguousarray(
                a.reshape(H_LOC, 128, T).transpose(1, 0, 2).reshape(128, H_LOC * T))

        in_maps.append({
            **xhi_p, **xlo_p,
            "wqhi": wqh, "wqlo": wql,
            "wkhi": wkh, "wklo": wkl,
            "wvhi": wvh, "wvlo": wvl,
            "wohi": pack_h(woT_hi), "wolo": pack_h(woT_lo),
            "cosT": cos_s,
            "sinT": sin_s,
            "skt": skt,
            "maskb": maskb,
            "ones_col": ones_col,
            "ones128": ones128,
            "ones2f8": ones2f8,
        })
    return in_maps


def kernel(x, w_ln, wq, wk, wv, wo, cos, sin):
    nc = _build_program()
    in_maps = _host_prep(x, w_ln, wq, wk, wv, wo, cos, sin)
    t0 = time.time()
    res = run_bass_kernel_spmd(nc, in_maps, core_ids=list(range(N_CORES)))
    t1 = time.time()
    print(f"run_bass_kernel_spmd wall: {(t1 - t0) * 1e3:.1f} ms", file=sys.stderr)
    acc = np.zeros((T, D), np.float32)
    for r in res.results:
        acc += np.asarray(r["out"], np.float32)
    return np.asarray(x, np.float32) + acc * (1.0 / OSC)
